# revision 1
# baseline (speedup 1.0000x reference)
"""Paged prefill attention (sparse_attention) on 8 Trainium2 NeuronCores.

Problem (hardcoded, mirrors the reference):
  q:        [2048, 32, 128] f32   (2 seqs x 1024 query tokens, 32 heads)
  k_cache:  [64, 64, 8, 128] f32  (64 physical blocks x 64 tokens x 8 kv heads)
  v_cache:  [64, 64, 8, 128] f32
  cu_seqlens_q: [0, 1024, 2048]
  cu_seqlens_k: [0, 2048, 4096]
  block_tables: [2, 32] int32 permutation of the 64 physical blocks
  out:      [2048, 32, 128] f32

Sharding: tensor-parallel by kv head. Core h gets kv head h plus its 4
query heads (GQA group 4), both full sequences. Each core runs the same
program (SPMD); the block-table gather is baked into the DMA descriptors
(the table is shared across heads, so one program serves all cores).

Per-core algorithm (S^T layout flash attention, fp16 matmuls):
  - K blocks are DMA-gathered per the block table, transposed on the PE
    (fp32), and stored as kT [d=128, tok] fp16.
  - Q tiles likewise transposed to qT [d=128, tok] fp16.
  - V chunks ([128 tok, 128 d]) are cast to fp16 with a ones column
    appended -> vP [128, 129] per chunk.
  - QK: S^T[k,q] = kT_tile.T @ qT, per 128-k-tile x 512-q-chunk, into
    PSUM, skipping fully-masked chunks (causal + 1024 history).
  - diagonal 128x128 tiles get an additive -1e10 upper-triangular mask.
  - exp(scale*s) on ScalarE straight from PSUM into an fp16 S^T buffer.
  - PV: for each 128-q tile, accumulate over k chunks
    out[q, 0:129] += expS_chunk.T @ vP_chunk  -- col 128 is the softmax
    denominator (ones column), cols 0:128 the unnormalized output.
  - normalize with VectorE reciprocal + per-partition scalar multiply,
    DMA out.
"""

import numpy as np

NUM_SEQS = 2
LQ = 1024
HIST = 1024
LK = LQ + HIST
NUM_HEADS = 32
NUM_KV_HEADS = 8
GROUP = NUM_HEADS // NUM_KV_HEADS  # 4 q heads per kv head / core
HEAD_DIM = 128
BLOCK_SIZE = 64
NBLK = LK // BLOCK_SIZE        # 32 logical blocks per sequence
TOTAL_BLOCKS = NUM_SEQS * NBLK  # 64 physical blocks
NCH = LK // 128                 # 16 128-token kv chunks per sequence
NQT = LQ // 128                 # 8 128-token q tiles per sequence
SCALE = 1.0 / float(np.sqrt(HEAD_DIM))
NEG = -1e10

_CACHE = {}


def _build_program(bt: np.ndarray):
    from contextlib import ExitStack

    import concourse.bass as bass
    import concourse.mybir as mybir
    import concourse.tile as tile
    from concourse import bacc
    from concourse.masks import make_identity

    f32 = mybir.dt.float32
    f16 = mybir.dt.float16

    nc = bacc.Bacc()
    q_d = nc.dram_tensor("q", [NUM_SEQS * LQ, GROUP, HEAD_DIM], f32,
                         kind="ExternalInput")
    k_d = nc.dram_tensor("k", [TOTAL_BLOCKS, BLOCK_SIZE, HEAD_DIM], f32,
                         kind="ExternalInput")
    v_d = nc.dram_tensor("v", [TOTAL_BLOCKS, BLOCK_SIZE, HEAD_DIM], f32,
                         kind="ExternalInput")
    o_d = nc.dram_tensor("out", [NUM_SEQS * LQ, GROUP, HEAD_DIM], f32,
                         kind="ExternalOutput")

    with tile.TileContext(nc) as tc, ExitStack() as ctx:
        consts = ctx.enter_context(tc.tile_pool(name="consts", bufs=1))
        persist = ctx.enter_context(tc.tile_pool(name="persist", bufs=1))
        stage = ctx.enter_context(tc.tile_pool(name="stage", bufs=4))
        small = ctx.enter_context(tc.tile_pool(name="small", bufs=4))
        es_pool = ctx.enter_context(tc.tile_pool(name="es", bufs=3))
        tp_ps = ctx.enter_context(tc.tile_pool(name="tp_ps", bufs=2, space="PSUM"))
        sc_ps = ctx.enter_context(tc.tile_pool(name="sc_ps", bufs=2, space="PSUM"))
        oc_ps = ctx.enter_context(tc.tile_pool(name="oc_ps", bufs=2, space="PSUM"))

        ident = consts.tile([128, 128], f32, tag="ident")
        make_identity(nc, ident[:, :])

        cmask = consts.tile([128, 128], f32, tag="cmask")
        nc.gpsimd.memset(cmask[:, :], 0.0)
        # keep (pass 0) where q_col >= k_row, else fill NEG
        nc.gpsimd.affine_select(
            out=cmask[:, :], in_=cmask[:, :],
            compare_op=mybir.AluOpType.is_ge, fill=NEG,
            base=0, pattern=[[1, 128]], channel_multiplier=-1,
        )

        qT = persist.tile([128, NUM_SEQS * GROUP * LQ], f16, tag="qT")
        kT = persist.tile([128, NUM_SEQS * LK], f16, tag="kT")
        vP = persist.tile([128, NUM_SEQS * NCH * 129], f16, tag="vP")

        def emit_kv(s):
            # ---- K / V load, gather, transpose (K), cast ----
            for c in range(NCH):  # chunk c = logical blocks 2c, 2c+1
                kst = stage.tile([128, 128], f32, tag="kst")
                vst = stage.tile([128, 128], f32, tag="vst")
                for half in range(2):
                    phys = int(bt[s, 2 * c + half])
                    nc.sync.dma_start(
                        out=kst[half * 64:(half + 1) * 64, :],
                        in_=k_d[phys, :, :])
                    nc.sync.dma_start(
                        out=vst[half * 64:(half + 1) * 64, :],
                        in_=v_d[phys, :, :])
                pst = tp_ps.tile([128, 128], f32, tag="tp")
                nc.tensor.transpose(pst[:, :], kst[:, :], ident[:, :])
                nc.vector.tensor_copy(
                    kT[:, s * LK + c * 128:s * LK + (c + 1) * 128], pst[:, :])
                base = (s * NCH + c) * 129
                nc.vector.tensor_copy(vP[:, base:base + 128], vst[:, :])
                nc.vector.memset(vP[:, base + 128:base + 129], 1.0)



        def emit_q(s, h):
            # ---- Q load + transpose ----
            qbase = (s * GROUP + h) * LQ
            for qt in range(NQT):
                qst = stage.tile([128, 128], f32, tag="qst")
                nc.sync.dma_start(
                    out=qst[:, :],
                    in_=q_d[s * LQ + qt * 128:s * LQ + (qt + 1) * 128, h, :])
                pst = tp_ps.tile([128, 128], f32, tag="tp")
                nc.tensor.transpose(pst[:, :], qst[:, :], ident[:, :])
                nc.vector.tensor_copy(
                    qT[:, qbase + qt * 128:qbase + (qt + 1) * 128],
                    pst[:, :])



        def emit_att(s, h):
            # ---- attention per (seq, head) ----
            qbase = (s * GROUP + h) * LQ
            es = es_pool.tile([128, NCH * LQ], f16, tag="es")
            for kt in range(NCH):
                        # exact causal clipping: query token i attends kv pos
                        # < HIST + i + 1, so chunk kt only needs q >= q_lo
                q_lo = max(0, (kt - NCH // 2) * 128)
                width = LQ - q_lo
                ps = sc_ps.tile([128, 1024], f32, tag="sc")
                off = 0
                while off < width:
                    n = min(512 - off % 512, width - off)
                    nc.tensor.matmul(
                        ps[:, off:off + n],
                        kT[:, s * LK + kt * 128:s * LK + (kt + 1) * 128],
                        qT[:, qbase + q_lo + off:qbase + q_lo + off + n],
                        start=True, stop=True)
                    off += n
                nc.scalar.activation(
                    es[:, kt * LQ + q_lo:(kt + 1) * LQ],
                    ps[:, 0:width],
                    mybir.ActivationFunctionType.Exp, scale=SCALE)
                if kt >= NCH // 2:
                    # zero strictly-lower-diagonal of the diag block on
                    # idle GPSIMD, off the PE->ACT critical chain
                    dc = kt * LQ + q_lo
                    nc.gpsimd.affine_select(
                        out=es[:, dc:dc + 128], in_=es[:, dc:dc + 128],
                        compare_op=mybir.AluOpType.is_ge, fill=0.0,
                        base=0, pattern=[[1, 128]], channel_multiplier=-1)
            for qt in range(NQT):
                nch_q = NCH // 2 + 1 + qt  # kv chunks 0 .. 8+qt
                po = oc_ps.tile([128, 129], f32, tag="oc")
                for c in range(nch_q):
                    nc.tensor.matmul(
                        po[:, :],
                        es[:, c * LQ + qt * 128:c * LQ + (qt + 1) * 128],
                        vP[:, (s * NCH + c) * 129:(s * NCH + c + 1) * 129],
                        start=(c == 0), stop=(c == nch_q - 1))
                rc = small.tile([128, 1], f32, tag="rc")
                nc.vector.reciprocal(rc[:, :], po[:, 128:129])
                ob = small.tile([128, 128], f32, tag="ob")
                nc.vector.tensor_scalar_mul(ob[:, :], po[:, 0:128], rc[:, :])
                nc.sync.dma_start(
                    out=o_d[s * LQ + qt * 128:s * LQ + (qt + 1) * 128, h, :],
                    in_=ob[:, :])



        emit_kv(0)
        for h in range(GROUP):
            emit_q(0, h)
        emit_att(0, 0)
        emit_kv(1)
        for h in range(GROUP):
            emit_q(1, h)
        for h in range(1, GROUP):
            emit_att(0, h)
        for h in range(GROUP):
            emit_att(1, h)

    nc.compile()
    return nc


def _get_program(bt: np.ndarray):
    key = bt.tobytes()
    if key not in _CACHE:
        _CACHE[key] = _build_program(bt)
    return _CACHE[key]


def kernel(q, k_cache, v_cache, cu_seqlens_q, cu_seqlens_k, block_tables,
           _want_trace=False):
    from concourse import bass_utils

    q = np.ascontiguousarray(np.asarray(q, dtype=np.float32))
    k_cache = np.ascontiguousarray(np.asarray(k_cache, dtype=np.float32))
    v_cache = np.ascontiguousarray(np.asarray(v_cache, dtype=np.float32))
    bt = np.asarray(block_tables, dtype=np.int32)

    assert q.shape == (NUM_SEQS * LQ, NUM_HEADS, HEAD_DIM)
    assert k_cache.shape == (TOTAL_BLOCKS, BLOCK_SIZE, NUM_KV_HEADS, HEAD_DIM)
    assert v_cache.shape == (TOTAL_BLOCKS, BLOCK_SIZE, NUM_KV_HEADS, HEAD_DIM)
    assert bt.shape == (NUM_SEQS, NBLK)
    assert bt.min() >= 0

    nc = _get_program(bt)

    in_maps = []
    for core in range(NUM_KV_HEADS):
        in_maps.append({
            "q": np.ascontiguousarray(
                q[:, core * GROUP:(core + 1) * GROUP, :]),
            "k": np.ascontiguousarray(k_cache[:, :, core, :]),
            "v": np.ascontiguousarray(v_cache[:, :, core, :]),
        })

    res = bass_utils.run_bass_kernel_spmd(
        nc, in_maps, core_ids=list(range(NUM_KV_HEADS)),
        trace=_want_trace,
        **({"trace_cores": list(range(NUM_KV_HEADS)), "stitch_traces": True}
           if _want_trace else {}),
    )

    out = np.empty((NUM_SEQS * LQ, NUM_HEADS, HEAD_DIM), dtype=np.float32)
    for core in range(NUM_KV_HEADS):
        out[:, core * GROUP:(core + 1) * GROUP, :] = res.results[core]["out"]

    if _want_trace:
        return out, res
    return out



# revision 2
# speedup vs baseline: 1.5933x; 1.5933x over previous
"""Paged prefill attention (sparse_attention) on 8 Trainium2 NeuronCores, v2.

Problem (hardcoded, mirrors the reference):
  q:        [2048, 32, 128] f32   (2 seqs x 1024 query tokens, 32 heads)
  k_cache:  [64, 64, 8, 128] f32  (64 physical blocks x 64 tokens x 8 kv heads)
  v_cache:  [64, 64, 8, 128] f32
  block_tables: [2, 32] int32 permutation of the 64 physical blocks
  out:      [2048, 32, 128] f32

Sharding: tensor-parallel by kv head. Core h gets kv head h plus its 4
query heads (GQA group 4), both full sequences.

v2 design vs the 250us baseline (which serialized 256 HWDGE DMA
descriptors at 625ns each):
  - K and V are loaded in PHYSICAL order with ONE dma_start each
    (partition = token%128 via a strided access pattern); the block-table
    permutation happens on-chip: K via per-half-block (64-partition) PE
    transposes into kT, V via partition-shifted DVE copies into vP.
  - Q: one DMA per (seq, head) = 8 total, [tok%128, qt, d] staging.
  - Output staged per seq as [q, 4*128] and written with one DMA per
    (seq, qtile) = 16 total.
  - Scores are computed in packed form: per (seq,head) pair the 16
    causally-clipped kv-chunk score panels (12800 columns total) are
    packed back-to-back into [128, 1536] PSUM fills -> 9 exp activation
    instructions per pair instead of 16 (ACT is the critical engine:
    0.833ns/col + ~185ns/instruction).
  - Software pipelining: fills (QK+exp) of pair p interleave with PV
    matmuls of pair p-1, keeping PE and ACT both ~100% busy; K/V/Q loads
    for seq 1 are sprinkled into early pair slots.
"""

import numpy as np

NUM_SEQS = 2
LQ = 1024
HIST = 1024
LK = LQ + HIST
NUM_HEADS = 32
NUM_KV_HEADS = 8
GROUP = NUM_HEADS // NUM_KV_HEADS  # 4 q heads per kv head / core
HEAD_DIM = 128
BLOCK_SIZE = 64
NBLK = LK // BLOCK_SIZE         # 32 logical blocks (64-token halves) per seq
NCH = LK // 128                 # 16 128-token kv chunks per sequence
NQT = LQ // 128                 # 8 128-token q tiles per sequence
SCALE = 1.0 / float(np.sqrt(HEAD_DIM))

# causal clipping per kv chunk kt: q columns [Q_LO[kt], 1024) are needed
Q_LO = [0] * 9 + [128 * i for i in range(1, 8)]
WIDTH = [LQ - lo for lo in Q_LO]
OFF = np.concatenate([[0], np.cumsum(WIDTH)]).tolist()  # packed col offsets
TOTAL_COLS = OFF[NCH]  # 12800
FILL = 1536            # packed columns per PSUM fill (3 banks)
NFILL = -(-TOTAL_COLS // FILL)  # 9

PAIRS = [(s, h) for s in range(NUM_SEQS) for h in range(GROUP)]

_CACHE = {}


def _plan_fills():
    """Static fill plan: for each fill, the packed window and the QK matmul
    segments (kt, packed_a, packed_b), split at chunk boundaries and at the
    512-col PSUM bank grid within the fill; plus which diag chunks' masks
    land wholly inside this fill."""
    fills = []
    for f in range(NFILL):
        lo, hi = f * FILL, min((f + 1) * FILL, TOTAL_COLS)
        segs = []
        for kt in range(NCH):
            a = max(OFF[kt], lo)
            b = min(OFF[kt + 1], hi)
            while a < b:
                # split at 512-grid relative to fill start
                nxt = lo + ((a - lo) // 512 + 1) * 512
                e = min(b, nxt)
                segs.append((kt, a, e))
                a = e
        diags = [kt for kt in range(NCH // 2, NCH)
                 if lo <= OFF[kt] and OFF[kt] + 128 <= hi]
        fills.append((lo, hi, segs, diags))
    return fills


FILLS = _plan_fills()


def _build_program(bt: np.ndarray):
    from contextlib import ExitStack

    import concourse.mybir as mybir
    import concourse.tile as tile
    from concourse import bacc
    from concourse.masks import make_identity

    f32 = mybir.dt.float32
    f16 = mybir.dt.float16

    nc = bacc.Bacc()
    q_d = nc.dram_tensor("q", [NUM_SEQS * LQ, GROUP, HEAD_DIM], f32,
                         kind="ExternalInput")
    k_d = nc.dram_tensor("k", [NUM_SEQS * LK, HEAD_DIM], f32,
                         kind="ExternalInput")
    v_d = nc.dram_tensor("v", [NUM_SEQS * LK, HEAD_DIM], f32,
                         kind="ExternalInput")
    o_d = nc.dram_tensor("out", [NUM_SEQS * LQ, GROUP, HEAD_DIM], f32,
                         kind="ExternalOutput")

    with tile.TileContext(nc) as tc, ExitStack() as ctx:
        consts = ctx.enter_context(tc.tile_pool(name="consts", bufs=1))
        kvstage = ctx.enter_context(tc.tile_pool(name="kvstage", bufs=1))
        qpool = ctx.enter_context(tc.tile_pool(name="qpool", bufs=2))
        persist = ctx.enter_context(tc.tile_pool(name="persist", bufs=1))
        espool = ctx.enter_context(tc.tile_pool(name="espool", bufs=2))
        opool = ctx.enter_context(tc.tile_pool(name="opool", bufs=2))
        small = ctx.enter_context(tc.tile_pool(name="small", bufs=4))
        scp = ctx.enter_context(tc.tile_pool(name="scp", bufs=2, space="PSUM"))
        ps1 = ctx.enter_context(tc.tile_pool(name="ps1", bufs=2, space="PSUM"))

        ident = consts.tile([128, 128], f32, tag="ident")
        make_identity(nc, ident[:, :])

        # persistent per-core data: kT [d, seq, chunk, tok], vP with ones col,
        # qT [d, seq, head, qcol]
        kT = persist.tile([128, NUM_SEQS, NCH, 128], f16, tag="kT")
        vP = persist.tile([128, NUM_SEQS * NCH, 129], f16, tag="vP")
        qT = persist.tile([128, NUM_SEQS, GROUP, LQ], f16, tag="qT")

        kst = kvstage.tile([128, NUM_SEQS * NCH, 128], f32, tag="kst")
        vst = kvstage.tile([128, NUM_SEQS * NCH, 128], f32, tag="vst")

        qstg = {}   # pair -> staging tile
        ostg = {}   # seq -> output staging tile
        es = {}     # pair -> packed exp-scores tile

        def kv_in_ap(t_d):
            # DRAM [4096, 128] enumerated (p, c, d) with row = c*128 + p
            return t_d[:, :].rearrange("(c p) d -> p c d", p=128)

        def emit_kv_dma():
            nc.sync.dma_start(out=kst[:, :, :], in_=kv_in_ap(k_d))
            nc.sync.dma_start(out=vst[:, :, :], in_=kv_in_ap(v_d))

        def emit_q_dma(p):
            s, h = PAIRS[p]
            qstg[p] = qpool.tile([128, NQT, 128], f32, tag="qstg",
                                 name=f"qstg{p}")
            in_ap = q_d[s * LQ:(s + 1) * LQ, h, :].rearrange(
                "(c p) d -> p c d", p=128)
            nc.sync.dma_start(out=qstg[p][:, :, :], in_=in_ap)

        def emit_kT_half(s, j):
            # logical half-block j of seq s -> kT[:, s, j//2, (j%2)*64 ...]
            b = int(bt[s, j])
            pt = ps1.tile([128, 129], f32, tag="ps1", name=f"tpk{s}_{j}")
            pb = (b % 2) * 64
            nc.tensor.transpose(
                pt[:, 0:64],
                kst[pb:pb + 64, b // 2, :],
                ident[pb:pb + 64, pb:pb + 64])
            c, h2 = j // 2, j % 2
            nc.vector.tensor_copy(
                kT[:, s, c, h2 * 64:(h2 + 1) * 64], pt[:, 0:64])

        def emit_vP_half(s, j):
            b = int(bt[s, j])
            c, h2 = j // 2, j % 2
            nc.vector.tensor_copy(
                vP[h2 * 64:(h2 + 1) * 64, s * NCH + c, 0:128],
                vst[(b % 2) * 64:(b % 2) * 64 + 64, b // 2, :])

        def emit_vP_ones(s, c):
            nc.vector.memset(vP[:, s * NCH + c, 128:129], 1.0)

        def emit_q_tr(p, qt):
            s, h = PAIRS[p]
            pt = ps1.tile([128, 129], f32, tag="ps1", name=f"tpq{p}_{qt}")
            nc.tensor.transpose(pt[:, 0:128], qstg[p][:, qt, :], ident[:, :])
            nc.vector.tensor_copy(
                qT[:, s, h, qt * 128:(qt + 1) * 128], pt[:, 0:128])

        def emit_fill(p, f):
            s, h = PAIRS[p]
            lo, hi, segs, diags = FILLS[f]
            sc = scp.tile([128, FILL], f32, tag="sc", name=f"sc{p}_{f}")
            for kt, a, b in segs:
                qcol = a - OFF[kt] + Q_LO[kt]
                nc.tensor.matmul(
                    sc[:, a - lo:b - lo],
                    kT[:, s, kt, :],
                    qT[:, s, h, qcol:qcol + (b - a)],
                    start=True, stop=True)
            nc.scalar.activation(
                es[p][:, lo:hi], sc[:, 0:hi - lo],
                mybir.ActivationFunctionType.Exp, scale=SCALE)
            for kt in diags:
                # zero the strictly-lower triangle of the diag panel
                dc = OFF[kt]
                nc.gpsimd.affine_select(
                    out=es[p][:, dc:dc + 128], in_=es[p][:, dc:dc + 128],
                    compare_op=mybir.AluOpType.is_ge, fill=0.0,
                    base=0, pattern=[[1, 128]], channel_multiplier=-1)

        def emit_pv(p, qt):
            s, h = PAIRS[p]
            nch_q = NCH // 2 + 1 + qt   # kv chunks 0 .. 8+qt
            po = ps1.tile([128, 129], f32, tag="ps1", name=f"po{p}_{qt}")
            for c in range(nch_q):
                col = OFF[c] + qt * 128 - Q_LO[c]
                nc.tensor.matmul(
                    po[:, :],
                    es[p][:, col:col + 128],
                    vP[:, s * NCH + c, :],
                    start=(c == 0), stop=(c == nch_q - 1))
            rc = small.tile([128, 1], f32, tag="rc", name=f"rc{p}_{qt}")
            nc.vector.reciprocal(rc[:, :], po[:, 128:129])
            nc.vector.tensor_scalar_mul(
                ostg[s][:, qt, h * 128:(h + 1) * 128], po[:, 0:128], rc[:, :])

        def emit_out_dma(s, qt):
            nc.sync.dma_start(
                out=o_d[s * LQ + qt * 128:s * LQ + (qt + 1) * 128, :, :],
                in_=ostg[s][:, qt, :])

        # ---------------- load phase (seq 0) ----------------
        emit_kv_dma()
        emit_q_dma(0)
        for j in range(NBLK):
            emit_kT_half(0, j)
        for qt in range(NQT):
            emit_q_tr(0, qt)
        for j in range(NBLK):
            emit_vP_half(0, j)
        for c in range(NCH):
            emit_vP_ones(0, c)
        emit_q_dma(1)

        # ---------------- pair 0 fills (+ pair-1 q transposes) ----------
        es[0] = espool.tile([128, TOTAL_COLS], f16, tag="es", name="es0")
        for f in range(NFILL):
            emit_fill(0, f)
            if f < NQT:
                emit_q_tr(1, f)

        # ---------------- steady pairs 1..7 ----------------
        for p in range(1, len(PAIRS)):
            s, h = PAIRS[p]
            if p == 1:
                ostg[0] = opool.tile([128, NQT, GROUP * 128], f32,
                                     tag="ostg", name="ostg0")
            if p == 5:
                ostg[1] = opool.tile([128, NQT, GROUP * 128], f32,
                                     tag="ostg", name="ostg1")
            if 1 <= p <= 6:
                emit_q_dma(p + 1)
            es[p] = espool.tile([128, TOTAL_COLS], f16, tag="es",
                                name=f"es{p}")
            for f in range(NFILL):
                emit_fill(p, f)
                if p == 1 and f < 8:
                    # sprinkle seq-1 kT half transposes, 4 per slot
                    for j in range(4 * f, 4 * (f + 1)):
                        emit_kT_half(1, j)
                if 1 <= p <= 6 and f < NQT:
                    emit_q_tr(p + 1, f)
                if f >= 1:
                    emit_pv(p - 1, f - 1)
                    if p == 4:
                        emit_out_dma(0, f - 1)
            if p == 1:
                for j in range(NBLK):
                    emit_vP_half(1, j)
                for c in range(NCH):
                    emit_vP_ones(1, c)
            del es[p - 1]

        # ---------------- tail: PV of last pair ----------------
        for qt in range(NQT):
            emit_pv(len(PAIRS) - 1, qt)
            emit_out_dma(1, qt)

    nc.compile()
    return nc


def _get_program(bt: np.ndarray):
    key = bt.tobytes()
    if key not in _CACHE:
        _CACHE[key] = _build_program(bt)
    return _CACHE[key]


def kernel(q, k_cache, v_cache, cu_seqlens_q, cu_seqlens_k, block_tables,
           _want_trace=False):
    from concourse import bass_utils

    q = np.ascontiguousarray(np.asarray(q, dtype=np.float32))
    k_cache = np.ascontiguousarray(np.asarray(k_cache, dtype=np.float32))
    v_cache = np.ascontiguousarray(np.asarray(v_cache, dtype=np.float32))
    bt = np.asarray(block_tables, dtype=np.int32)

    assert q.shape == (NUM_SEQS * LQ, NUM_HEADS, HEAD_DIM)
    assert k_cache.shape == (NUM_SEQS * NBLK, BLOCK_SIZE, NUM_KV_HEADS,
                             HEAD_DIM)
    assert v_cache.shape == k_cache.shape
    assert bt.shape == (NUM_SEQS, NBLK)
    assert bt.min() >= 0

    nc = _get_program(bt)

    in_maps = []
    for core in range(NUM_KV_HEADS):
        in_maps.append({
            "q": np.ascontiguousarray(
                q[:, core * GROUP:(core + 1) * GROUP, :]),
            "k": np.ascontiguousarray(
                k_cache[:, :, core, :]).reshape(NUM_SEQS * LK, HEAD_DIM),
            "v": np.ascontiguousarray(
                v_cache[:, :, core, :]).reshape(NUM_SEQS * LK, HEAD_DIM),
        })

    res = bass_utils.run_bass_kernel_spmd(
        nc, in_maps, core_ids=list(range(NUM_KV_HEADS)),
        trace=_want_trace,
        **({"trace_cores": list(range(NUM_KV_HEADS)), "stitch_traces": True}
           if _want_trace else {}),
    )

    out = np.empty((NUM_SEQS * LQ, NUM_HEADS, HEAD_DIM), dtype=np.float32)
    for core in range(NUM_KV_HEADS):
        out[:, core * GROUP:(core + 1) * GROUP, :] = res.results[core]["out"]

    if _want_trace:
        return out, res
    return out


# revision 4
# speedup vs baseline: 1.6828x; 1.0562x over previous
"""Paged prefill attention (sparse_attention) on 8 Trainium2 NeuronCores, v2.

Problem (hardcoded, mirrors the reference):
  q:        [2048, 32, 128] f32   (2 seqs x 1024 query tokens, 32 heads)
  k_cache:  [64, 64, 8, 128] f32  (64 physical blocks x 64 tokens x 8 kv heads)
  v_cache:  [64, 64, 8, 128] f32
  block_tables: [2, 32] int32 permutation of the 64 physical blocks
  out:      [2048, 32, 128] f32

Sharding: tensor-parallel by kv head. Core h gets kv head h plus its 4
query heads (GQA group 4), both full sequences.

v2 design vs the 250us baseline (which serialized 256 HWDGE DMA
descriptors at 625ns each):
  - K and V are loaded in PHYSICAL order with ONE dma_start each
    (partition = token%128 via a strided access pattern); the block-table
    permutation happens on-chip: K via per-half-block (64-partition) PE
    transposes into kT, V via partition-shifted DVE copies into vP.
  - Q: one DMA per (seq, head) = 8 total, [tok%128, qt, d] staging.
  - Output staged per seq as [q, 4*128] and written with one DMA per
    (seq, qtile) = 16 total.
  - Scores are computed in packed form: per (seq,head) pair the 16
    causally-clipped kv-chunk score panels (12800 columns total) are
    packed back-to-back into [128, 1536] PSUM fills -> 9 exp activation
    instructions per pair instead of 16 (ACT is the critical engine:
    0.833ns/col + ~185ns/instruction).
  - Software pipelining: fills (QK+exp) of pair p interleave with PV
    matmuls of pair p-1, keeping PE and ACT both ~100% busy; K/V/Q loads
    for seq 1 are sprinkled into early pair slots.
"""

import numpy as np

NUM_SEQS = 2
LQ = 1024
HIST = 1024
LK = LQ + HIST
NUM_HEADS = 32
NUM_KV_HEADS = 8
GROUP = NUM_HEADS // NUM_KV_HEADS  # 4 q heads per kv head / core
HEAD_DIM = 128
BLOCK_SIZE = 64
NBLK = LK // BLOCK_SIZE         # 32 logical blocks (64-token halves) per seq
NCH = LK // 128                 # 16 128-token kv chunks per sequence
NQT = LQ // 128                 # 8 128-token q tiles per sequence
SCALE = 1.0 / float(np.sqrt(HEAD_DIM))

# causal clipping per kv chunk kt: q columns [Q_LO[kt], 1024) are needed
Q_LO = [0] * 9 + [128 * i for i in range(1, 8)]
WIDTH = [LQ - lo for lo in Q_LO]
OFF = np.concatenate([[0], np.cumsum(WIDTH)]).tolist()  # packed col offsets
TOTAL_COLS = OFF[NCH]  # 12800
FILL = 1536            # packed columns per PSUM fill (3 banks)
NFILL = -(-TOTAL_COLS // FILL)  # 9

PAIRS = [(s, h) for s in range(NUM_SEQS) for h in range(GROUP)]

_CACHE = {}


def _plan_fills():
    """Static fill plan: for each fill, the packed window and the QK matmul
    segments (kt, packed_a, packed_b), split at chunk boundaries and at the
    512-col PSUM bank grid within the fill; plus which diag chunks' masks
    land wholly inside this fill."""
    fills = []
    for f in range(NFILL):
        lo, hi = f * FILL, min((f + 1) * FILL, TOTAL_COLS)
        segs = []
        for kt in range(NCH):
            a = max(OFF[kt], lo)
            b = min(OFF[kt + 1], hi)
            while a < b:
                # split at 512-grid relative to fill start
                nxt = lo + ((a - lo) // 512 + 1) * 512
                e = min(b, nxt)
                segs.append((kt, a, e))
                a = e
        diags = [kt for kt in range(NCH // 2, NCH)
                 if lo <= OFF[kt] and OFF[kt] + 128 <= hi]
        fills.append((lo, hi, segs, diags))
    return fills


FILLS = _plan_fills()


def _build_program(bt: np.ndarray):
    from contextlib import ExitStack

    import concourse.mybir as mybir
    import concourse.tile as tile
    from concourse import bacc
    from concourse.masks import make_identity

    f32 = mybir.dt.float32
    f16 = mybir.dt.float16

    nc = bacc.Bacc()
    q_d = nc.dram_tensor("q", [NUM_SEQS * LQ, GROUP, HEAD_DIM], f32,
                         kind="ExternalInput")
    k_d = nc.dram_tensor("k", [NUM_SEQS * LK, HEAD_DIM], f32,
                         kind="ExternalInput")
    v_d = nc.dram_tensor("v", [NUM_SEQS * LK, HEAD_DIM], f32,
                         kind="ExternalInput")
    o_d = nc.dram_tensor("out", [NUM_SEQS * LQ, GROUP, HEAD_DIM], f32,
                         kind="ExternalOutput")

    with tile.TileContext(nc) as tc, ExitStack() as ctx:
        consts = ctx.enter_context(tc.tile_pool(name="consts", bufs=1))
        kvstage = ctx.enter_context(tc.tile_pool(name="kvstage", bufs=1))
        qpool = ctx.enter_context(tc.tile_pool(name="qpool", bufs=4))
        persist = ctx.enter_context(tc.tile_pool(name="persist", bufs=1))
        espool = ctx.enter_context(tc.tile_pool(name="espool", bufs=3))
        opool = ctx.enter_context(tc.tile_pool(name="opool", bufs=2))
        small = ctx.enter_context(tc.tile_pool(name="small", bufs=4))
        scp = ctx.enter_context(tc.tile_pool(name="scp", bufs=2, space="PSUM"))
        ps1 = ctx.enter_context(tc.tile_pool(name="ps1", bufs=2, space="PSUM"))

        ident = consts.tile([128, 128], f32, tag="ident")
        make_identity(nc, ident[:, :])

        # persistent per-core data: kT [d, seq, chunk, tok], vP with ones col,
        # qT [d, seq, head, qcol]
        kT = persist.tile([128, NUM_SEQS, NCH, 128], f16, tag="kT")
        vP = persist.tile([128, NUM_SEQS * NCH, 129], f16, tag="vP")
        qT = persist.tile([128, NUM_SEQS, GROUP, LQ], f16, tag="qT")

        kst = kvstage.tile([128, NUM_SEQS * NCH, 128], f32, tag="kst")
        vst = kvstage.tile([128, NUM_SEQS * NCH, 128], f32, tag="vst")

        qstg = {}   # pair -> staging tile
        ostg = {}   # seq -> output staging tile
        es = {}     # pair -> packed exp-scores tile

        def kv_in_ap(t_d, s):
            # DRAM seq-s half [2048, 128] enumerated (p, c, d), row = c*128+p
            return t_d[s * LK:(s + 1) * LK, :].rearrange(
                "(c p) d -> p c d", p=128)

        def emit_kv_dma(t_d, st, s):
            nc.sync.dma_start(out=st[:, s * NCH:(s + 1) * NCH, :],
                              in_=kv_in_ap(t_d, s))

        def emit_q_dma(p):
            s, h = PAIRS[p]
            qstg[p] = qpool.tile([128, NQT, 128], f32, tag="qstg",
                                 name=f"qstg{p}")
            in_ap = q_d[s * LQ:(s + 1) * LQ, h, :].rearrange(
                "(c p) d -> p c d", p=128)
            nc.sync.dma_start(out=qstg[p][:, :, :], in_=in_ap)

        def emit_kT_chunk(s, c):
            # logical chunk c of seq s: each 64-token half-block transposed
            # into its own PSUM tile (device rejects two transpose groups
            # sharing one PSUM tile)
            for h2 in range(2):
                b = int(bt[s, 2 * c + h2])
                pb = (b % 2) * 64
                pt = ps1.tile([128, 129], f32, tag="ps1",
                              name=f"tpk{s}_{c}_{h2}")
                nc.tensor.transpose(
                    pt[:, 0:64],
                    kst[pb:pb + 64, b // 2, :],
                    ident[pb:pb + 64, pb:pb + 64])
                nc.vector.tensor_copy(
                    kT[:, s, c, h2 * 64:(h2 + 1) * 64], pt[:, 0:64])

        def emit_vP_half(s, j):
            b = int(bt[s, j])
            c, h2 = j // 2, j % 2
            nc.vector.tensor_copy(
                vP[h2 * 64:(h2 + 1) * 64, s * NCH + c, 0:128],
                vst[(b % 2) * 64:(b % 2) * 64 + 64, b // 2, :])

        def emit_vP_ones(s, c):
            nc.vector.memset(vP[:, s * NCH + c, 128:129], 1.0)

        def emit_q_tr(p, qt):
            s, h = PAIRS[p]
            pt = ps1.tile([128, 129], f32, tag="ps1", name=f"tpq{p}_{qt}")
            nc.tensor.transpose(pt[:, 0:128], qstg[p][:, qt, :], ident[:, :])
            nc.vector.tensor_copy(
                qT[:, s, h, qt * 128:(qt + 1) * 128], pt[:, 0:128])

        def emit_fill(p, f):
            s, h = PAIRS[p]
            lo, hi, segs, diags = FILLS[f]
            sc = scp.tile([128, FILL], f32, tag="sc", name=f"sc{p}_{f}")
            for kt, a, b in segs:
                qcol = a - OFF[kt] + Q_LO[kt]
                nc.tensor.matmul(
                    sc[:, a - lo:b - lo],
                    kT[:, s, kt, :],
                    qT[:, s, h, qcol:qcol + (b - a)],
                    start=True, stop=True)
            nc.scalar.activation(
                es[p][:, lo:hi], sc[:, 0:hi - lo],
                mybir.ActivationFunctionType.Exp, scale=SCALE)
            for kt in diags:
                # zero the strictly-lower triangle of the diag panel
                dc = OFF[kt]
                nc.gpsimd.affine_select(
                    out=es[p][:, dc:dc + 128], in_=es[p][:, dc:dc + 128],
                    compare_op=mybir.AluOpType.is_ge, fill=0.0,
                    base=0, pattern=[[1, 128]], channel_multiplier=-1)

        def emit_pv(p, qt):
            s, h = PAIRS[p]
            nch_q = NCH // 2 + 1 + qt   # kv chunks 0 .. 8+qt
            po = ps1.tile([128, 129], f32, tag="ps1", name=f"po{p}_{qt}")
            for c in range(nch_q):
                col = OFF[c] + qt * 128 - Q_LO[c]
                nc.tensor.matmul(
                    po[:, :],
                    es[p][:, col:col + 128],
                    vP[:, s * NCH + c, :],
                    start=(c == 0), stop=(c == nch_q - 1))
            rc = small.tile([128, 1], f32, tag="rc", name=f"rc{p}_{qt}")
            nc.vector.reciprocal(rc[:, :], po[:, 128:129])
            nc.vector.tensor_scalar_mul(
                ostg[s][:, qt, h * 128:(h + 1) * 128], po[:, 0:128], rc[:, :])

        def emit_out_dma(s, qt):
            nc.sync.dma_start(
                out=o_d[s * LQ + qt * 128:s * LQ + (qt + 1) * 128, :, :],
                in_=ostg[s][:, qt, :])

        # per-fill kT-chunk prerequisites (cumulative)
        chunks_needed = [max(kt for kt, _, _ in FILLS[f][2]) + 1
                         for f in range(NFILL)]

        # ---------------- load phase ----------------
        # DMA issue order gates the (shared, serial) DMA engines: K first
        # (kT feeds the first QK fills), then q0/q1, then V (vP is only
        # needed once PV of pair 0 starts, one pair-time later).
        emit_q_dma(0)
        emit_kv_dma(k_d, kst, 0)
        emit_kv_dma(k_d, kst, 1)
        emit_q_dma(1)
        emit_kv_dma(v_d, vst, 0)
        emit_kv_dma(v_d, vst, 1)

        # ---------------- pair 0: fills + just-in-time transposes -------
        es[0] = espool.tile([128, TOTAL_COLS], f16, tag="es", name="es0")
        for qt in range(NQT):
            emit_q_tr(0, qt)
        done_chunks = 0
        for f in range(NFILL):
            while done_chunks < chunks_needed[f]:
                emit_kT_chunk(0, done_chunks)
                done_chunks += 1
            emit_fill(0, f)
            if 4 <= f <= 7:
                emit_q_tr(1, f - 4)
                emit_q_tr(1, f)
        while done_chunks < NCH:
            emit_kT_chunk(0, done_chunks)
            done_chunks += 1
        # vP for seq 0 (DVE-only block; V DMA has landed by now)
        for j in range(NBLK):
            emit_vP_half(0, j)
        for c in range(NCH):
            emit_vP_ones(0, c)

        # ---------------- steady pairs 1..7 ----------------
        LAST = len(PAIRS) - 1
        for p in range(1, len(PAIRS)):
            s, h = PAIRS[p]
            if p == 1:
                ostg[0] = opool.tile([128, NQT, GROUP * 128], f32,
                                     tag="ostg", name="ostg0")
            if p == 5:
                ostg[1] = opool.tile([128, NQT, GROUP * 128], f32,
                                     tag="ostg", name="ostg1")
            if 1 <= p <= 6:
                emit_q_dma(p + 1)
            es[p] = espool.tile([128, TOTAL_COLS], f16, tag="es",
                                name=f"es{p}")
            if p < LAST:
                # fills of pair p interleave PV batches of pair p-1
                for f in range(NFILL):
                    emit_fill(p, f)
                    if p in (1, 2) and f >= 3:
                        # seq-1 kT chunks: 6 per pair over pairs 1-2 (f3..f8)
                        emit_kT_chunk(1, (p - 1) * 6 + (f - 3))
                    if p == 3 and 5 <= f <= 8:
                        emit_kT_chunk(1, 12 + (f - 5))
                    if p in (4, 5) and f < 8:
                        # seq-1 vP copies, 2 per slot over pairs 4-5 (DVE)
                        base = (p - 4) * 16 + 2 * f
                        emit_vP_half(1, base)
                        emit_vP_half(1, base + 1)
                        emit_vP_ones(1, (p - 4) * 8 + f)
                    if 1 <= p <= 6 and f < NQT:
                        emit_q_tr(p + 1, f)
                    if f >= 1:
                        emit_pv(p - 1, f - 1)
                        if p == 4:
                            emit_out_dma(0, f - 1)
            else:
                # last pair: compress PV(p-1) into early slots and start
                # PV(p) as soon as its enabling fill (and diag mask) lands
                for f in range(NFILL):
                    emit_fill(p, f)
                    if f == 0:
                        emit_pv(p - 1, 0)
                    elif f <= 3:
                        emit_pv(p - 1, 2 * f - 1)
                        emit_pv(p - 1, 2 * f)
                    elif f == 4:
                        emit_pv(p - 1, 7)
                    # PV(p, qt) ready after fill 5 (qt0), 6 (qt1,2),
                    # 7 (qt3..5), 8 (qt6,7)
                    if f == 5:
                        emit_pv(p, 0)
                    elif f == 6:
                        emit_pv(p, 1)
                        emit_pv(p, 2)
                    elif f == 7:
                        emit_pv(p, 3)
                        emit_pv(p, 4)
                    elif f == 8:
                        emit_pv(p, 5)
                for qt in range(NQT):
                    if qt >= 6:
                        emit_pv(p, qt)
                    emit_out_dma(1, qt)
            if p >= 2:
                del es[p - 2]

    nc.compile()
    return nc


def _get_program(bt: np.ndarray):
    key = bt.tobytes()
    if key not in _CACHE:
        _CACHE[key] = _build_program(bt)
    return _CACHE[key]


def kernel(q, k_cache, v_cache, cu_seqlens_q, cu_seqlens_k, block_tables,
           _want_trace=False):
    from concourse import bass_utils

    q = np.ascontiguousarray(np.asarray(q, dtype=np.float32))
    k_cache = np.ascontiguousarray(np.asarray(k_cache, dtype=np.float32))
    v_cache = np.ascontiguousarray(np.asarray(v_cache, dtype=np.float32))
    bt = np.asarray(block_tables, dtype=np.int32)

    assert q.shape == (NUM_SEQS * LQ, NUM_HEADS, HEAD_DIM)
    assert k_cache.shape == (NUM_SEQS * NBLK, BLOCK_SIZE, NUM_KV_HEADS,
                             HEAD_DIM)
    assert v_cache.shape == k_cache.shape
    assert bt.shape == (NUM_SEQS, NBLK)
    assert bt.min() >= 0

    nc = _get_program(bt)

    in_maps = []
    for core in range(NUM_KV_HEADS):
        in_maps.append({
            "q": np.ascontiguousarray(
                q[:, core * GROUP:(core + 1) * GROUP, :]),
            "k": np.ascontiguousarray(
                k_cache[:, :, core, :]).reshape(NUM_SEQS * LK, HEAD_DIM),
            "v": np.ascontiguousarray(
                v_cache[:, :, core, :]).reshape(NUM_SEQS * LK, HEAD_DIM),
        })

    res = bass_utils.run_bass_kernel_spmd(
        nc, in_maps, core_ids=list(range(NUM_KV_HEADS)),
        trace=_want_trace,
        **({"trace_cores": list(range(NUM_KV_HEADS)), "stitch_traces": True}
           if _want_trace else {}),
    )

    out = np.empty((NUM_SEQS * LQ, NUM_HEADS, HEAD_DIM), dtype=np.float32)
    for core in range(NUM_KV_HEADS):
        out[:, core * GROUP:(core + 1) * GROUP, :] = res.results[core]["out"]

    if _want_trace:
        return out, res
    return out


# revision 5
# speedup vs baseline: 1.6848x; 1.0012x over previous
"""Paged prefill attention (sparse_attention) on 8 Trainium2 NeuronCores, v2.

Problem (hardcoded, mirrors the reference):
  q:        [2048, 32, 128] f32   (2 seqs x 1024 query tokens, 32 heads)
  k_cache:  [64, 64, 8, 128] f32  (64 physical blocks x 64 tokens x 8 kv heads)
  v_cache:  [64, 64, 8, 128] f32
  block_tables: [2, 32] int32 permutation of the 64 physical blocks
  out:      [2048, 32, 128] f32

Sharding: tensor-parallel by kv head. Core h gets kv head h plus its 4
query heads (GQA group 4), both full sequences.

v2 design vs the 250us baseline (which serialized 256 HWDGE DMA
descriptors at 625ns each):
  - K and V are loaded in PHYSICAL order with ONE dma_start each
    (partition = token%128 via a strided access pattern); the block-table
    permutation happens on-chip: K via per-half-block (64-partition) PE
    transposes into kT, V via partition-shifted DVE copies into vP.
  - Q: one DMA per (seq, head) = 8 total, [tok%128, qt, d] staging.
  - Output staged per seq as [q, 4*128] and written with one DMA per
    (seq, qtile) = 16 total.
  - Scores are computed in packed form: per (seq,head) pair the 16
    causally-clipped kv-chunk score panels (12800 columns total) are
    packed back-to-back into [128, 1536] PSUM fills -> 9 exp activation
    instructions per pair instead of 16 (ACT is the critical engine:
    0.833ns/col + ~185ns/instruction).
  - Software pipelining: fills (QK+exp) of pair p interleave with PV
    matmuls of pair p-1, keeping PE and ACT both ~100% busy; K/V/Q loads
    for seq 1 are sprinkled into early pair slots.
"""

import numpy as np

NUM_SEQS = 2
LQ = 1024
HIST = 1024
LK = LQ + HIST
NUM_HEADS = 32
NUM_KV_HEADS = 8
GROUP = NUM_HEADS // NUM_KV_HEADS  # 4 q heads per kv head / core
HEAD_DIM = 128
BLOCK_SIZE = 64
NBLK = LK // BLOCK_SIZE         # 32 logical blocks (64-token halves) per seq
NCH = LK // 128                 # 16 128-token kv chunks per sequence
NQT = LQ // 128                 # 8 128-token q tiles per sequence
SCALE = 1.0 / float(np.sqrt(HEAD_DIM))

# causal clipping per kv chunk kt: q columns [Q_LO[kt], 1024) are needed
Q_LO = [0] * 9 + [128 * i for i in range(1, 8)]
WIDTH = [LQ - lo for lo in Q_LO]
OFF = np.concatenate([[0], np.cumsum(WIDTH)]).tolist()  # packed col offsets
TOTAL_COLS = OFF[NCH]  # 12800
FILL = 1536            # packed columns per PSUM fill (3 banks)
NFILL = -(-TOTAL_COLS // FILL)  # 9

PAIRS = [(s, h) for s in range(NUM_SEQS) for h in range(GROUP)]

_CACHE = {}


def _plan_fills():
    """Static fill plan: for each fill, the packed window and the QK matmul
    segments (kt, packed_a, packed_b), split at chunk boundaries and at the
    512-col PSUM bank grid within the fill; plus which diag chunks' masks
    land wholly inside this fill."""
    fills = []
    for f in range(NFILL):
        lo, hi = f * FILL, min((f + 1) * FILL, TOTAL_COLS)
        segs = []
        for kt in range(NCH):
            a = max(OFF[kt], lo)
            b = min(OFF[kt + 1], hi)
            while a < b:
                # split at 512-grid relative to fill start
                nxt = lo + ((a - lo) // 512 + 1) * 512
                e = min(b, nxt)
                segs.append((kt, a, e))
                a = e
        diags = [kt for kt in range(NCH // 2, NCH)
                 if lo <= OFF[kt] and OFF[kt] + 128 <= hi]
        fills.append((lo, hi, segs, diags))
    return fills


FILLS = _plan_fills()


def _build_program(bt: np.ndarray):
    from contextlib import ExitStack

    import concourse.mybir as mybir
    import concourse.tile as tile
    from concourse import bacc
    from concourse.masks import make_identity

    f32 = mybir.dt.float32
    f16 = mybir.dt.float16

    nc = bacc.Bacc()
    q_d = nc.dram_tensor("q", [NUM_SEQS * LQ, GROUP, HEAD_DIM], f32,
                         kind="ExternalInput")
    k_d = nc.dram_tensor("k", [NUM_SEQS * LK, HEAD_DIM], f32,
                         kind="ExternalInput")
    v_d = nc.dram_tensor("v", [NUM_SEQS * LK, HEAD_DIM], f32,
                         kind="ExternalInput")
    o_d = nc.dram_tensor("out", [NUM_SEQS * LQ, GROUP, HEAD_DIM], f32,
                         kind="ExternalOutput")

    with tile.TileContext(nc) as tc, ExitStack() as ctx:
        consts = ctx.enter_context(tc.tile_pool(name="consts", bufs=1))
        kvstage = ctx.enter_context(tc.tile_pool(name="kvstage", bufs=1))
        qpool = ctx.enter_context(tc.tile_pool(name="qpool", bufs=4))
        persist = ctx.enter_context(tc.tile_pool(name="persist", bufs=1))
        espool = ctx.enter_context(tc.tile_pool(name="espool", bufs=3))
        opool = ctx.enter_context(tc.tile_pool(name="opool", bufs=2))
        small = ctx.enter_context(tc.tile_pool(name="small", bufs=4))
        scp = ctx.enter_context(tc.tile_pool(name="scp", bufs=2, space="PSUM"))
        ps1 = ctx.enter_context(tc.tile_pool(name="ps1", bufs=2, space="PSUM"))

        ident = consts.tile([128, 128], f32, tag="ident")
        make_identity(nc, ident[:, :])

        # persistent per-core data: kT [d, seq, chunk, tok], vP with ones col,
        # qT [d, seq, head, qcol]
        kT = persist.tile([128, NUM_SEQS, NCH, 128], f16, tag="kT")
        vP = persist.tile([128, NUM_SEQS * NCH, 129], f16, tag="vP")
        qT = persist.tile([128, NUM_SEQS, GROUP, LQ], f16, tag="qT")

        kst = kvstage.tile([128, NUM_SEQS * NCH, 128], f32, tag="kst")
        vst = kvstage.tile([128, NUM_SEQS * NCH, 128], f32, tag="vst")

        qstg = {}   # pair -> staging tile
        ostg = {}   # seq -> output staging tile
        es = {}     # pair -> packed exp-scores tile

        def kv_in_ap(t_d, s):
            # DRAM seq-s half [2048, 128] enumerated (p, c, d), row = c*128+p
            return t_d[s * LK:(s + 1) * LK, :].rearrange(
                "(c p) d -> p c d", p=128)

        def emit_kv_dma(t_d, st, s):
            nc.sync.dma_start(out=st[:, s * NCH:(s + 1) * NCH, :],
                              in_=kv_in_ap(t_d, s))

        def emit_q_dma(p):
            s, h = PAIRS[p]
            qstg[p] = qpool.tile([128, NQT, 128], f32, tag="qstg",
                                 name=f"qstg{p}")
            in_ap = q_d[s * LQ:(s + 1) * LQ, h, :].rearrange(
                "(c p) d -> p c d", p=128)
            nc.sync.dma_start(out=qstg[p][:, :, :], in_=in_ap)

        def emit_kT_chunk(s, c):
            # logical chunk c of seq s: each 64-token half-block transposed
            # into its own PSUM tile (device rejects two transpose groups
            # sharing one PSUM tile)
            for h2 in range(2):
                b = int(bt[s, 2 * c + h2])
                pb = (b % 2) * 64
                pt = ps1.tile([128, 129], f32, tag="ps1",
                              name=f"tpk{s}_{c}_{h2}")
                nc.tensor.transpose(
                    pt[:, 0:64],
                    kst[pb:pb + 64, b // 2, :],
                    ident[pb:pb + 64, pb:pb + 64])
                nc.vector.tensor_copy(
                    kT[:, s, c, h2 * 64:(h2 + 1) * 64], pt[:, 0:64])

        def emit_vP_half(s, j):
            b = int(bt[s, j])
            c, h2 = j // 2, j % 2
            nc.vector.tensor_copy(
                vP[h2 * 64:(h2 + 1) * 64, s * NCH + c, 0:128],
                vst[(b % 2) * 64:(b % 2) * 64 + 64, b // 2, :])

        def emit_vP_ones(s, c):
            nc.vector.memset(vP[:, s * NCH + c, 128:129], 1.0)

        def emit_q_tr(p, qt):
            s, h = PAIRS[p]
            pt = ps1.tile([128, 129], f32, tag="ps1", name=f"tpq{p}_{qt}")
            nc.tensor.transpose(pt[:, 0:128], qstg[p][:, qt, :], ident[:, :])
            nc.vector.tensor_copy(
                qT[:, s, h, qt * 128:(qt + 1) * 128], pt[:, 0:128])

        def emit_fill(p, f):
            s, h = PAIRS[p]
            lo, hi, segs, diags = FILLS[f]
            sc = scp.tile([128, FILL], f32, tag="sc", name=f"sc{p}_{f}")
            for kt, a, b in segs:
                qcol = a - OFF[kt] + Q_LO[kt]
                nc.tensor.matmul(
                    sc[:, a - lo:b - lo],
                    kT[:, s, kt, :],
                    qT[:, s, h, qcol:qcol + (b - a)],
                    start=True, stop=True)
            nc.scalar.activation(
                es[p][:, lo:hi], sc[:, 0:hi - lo],
                mybir.ActivationFunctionType.Exp, scale=SCALE)
            for kt in diags:
                # zero the strictly-lower triangle of the diag panel
                dc = OFF[kt]
                nc.gpsimd.affine_select(
                    out=es[p][:, dc:dc + 128], in_=es[p][:, dc:dc + 128],
                    compare_op=mybir.AluOpType.is_ge, fill=0.0,
                    base=0, pattern=[[1, 128]], channel_multiplier=-1)

        def emit_pv(p, qt, pool=None):
            s, h = PAIRS[p]
            nch_q = NCH // 2 + 1 + qt   # kv chunks 0 .. 8+qt
            if pool is None:
                po = ps1.tile([128, 129], f32, tag="ps1", name=f"po{p}_{qt}")
            else:
                po = pool.tile([128, 129], f32, tag="sc", name=f"po{p}_{qt}")
            for c in range(nch_q):
                col = OFF[c] + qt * 128 - Q_LO[c]
                nc.tensor.matmul(
                    po[:, :],
                    es[p][:, col:col + 128],
                    vP[:, s * NCH + c, :],
                    start=(c == 0), stop=(c == nch_q - 1))
            rc = small.tile([128, 1], f32, tag="rc", name=f"rc{p}_{qt}")
            nc.vector.reciprocal(rc[:, :], po[:, 128:129])
            nc.vector.tensor_scalar_mul(
                ostg[s][:, qt, h * 128:(h + 1) * 128], po[:, 0:128], rc[:, :])

        def emit_out_dma(s, qt):
            nc.sync.dma_start(
                out=o_d[s * LQ + qt * 128:s * LQ + (qt + 1) * 128, :, :],
                in_=ostg[s][:, qt, :])

        # per-fill kT-chunk prerequisites (cumulative)
        chunks_needed = [max(kt for kt, _, _ in FILLS[f][2]) + 1
                         for f in range(NFILL)]

        # ---------------- load phase ----------------
        # DMA issue order gates the (shared, serial) DMA engines: K first
        # (kT feeds the first QK fills), then q0/q1, then V (vP is only
        # needed once PV of pair 0 starts, one pair-time later).
        emit_q_dma(0)
        emit_kv_dma(k_d, kst, 0)
        emit_kv_dma(k_d, kst, 1)
        emit_q_dma(1)
        emit_kv_dma(v_d, vst, 0)
        emit_kv_dma(v_d, vst, 1)

        # ---------------- pair 0: fills + just-in-time transposes -------
        es[0] = espool.tile([128, TOTAL_COLS], f16, tag="es", name="es0")
        for qt in range(NQT):
            emit_q_tr(0, qt)
        done_chunks = 0
        for f in range(NFILL):
            while done_chunks < chunks_needed[f]:
                emit_kT_chunk(0, done_chunks)
                done_chunks += 1
            emit_fill(0, f)
            if 4 <= f <= 7:
                emit_q_tr(1, f - 4)
                emit_q_tr(1, f)
        while done_chunks < NCH:
            emit_kT_chunk(0, done_chunks)
            done_chunks += 1
        # vP for seq 0 (DVE-only block; V DMA has landed by now)
        for j in range(NBLK):
            emit_vP_half(0, j)
        for c in range(NCH):
            emit_vP_ones(0, c)

        # ---------------- steady pairs 1..7 ----------------
        LAST = len(PAIRS) - 1
        for p in range(1, len(PAIRS)):
            s, h = PAIRS[p]
            if p == 1:
                ostg[0] = opool.tile([128, NQT, GROUP * 128], f32,
                                     tag="ostg", name="ostg0")
            if p == 5:
                ostg[1] = opool.tile([128, NQT, GROUP * 128], f32,
                                     tag="ostg", name="ostg1")
            if 1 <= p <= 6:
                emit_q_dma(p + 1)
            es[p] = espool.tile([128, TOTAL_COLS], f16, tag="es",
                                name=f"es{p}")
            if p < LAST:
                # fills of pair p interleave PV batches of pair p-1
                for f in range(NFILL):
                    emit_fill(p, f)
                    if p in (1, 2) and f >= 3:
                        # seq-1 kT chunks: 6 per pair over pairs 1-2 (f3..f8)
                        emit_kT_chunk(1, (p - 1) * 6 + (f - 3))
                    if p == 3 and 5 <= f <= 8:
                        emit_kT_chunk(1, 12 + (f - 5))
                    if p in (4, 5) and f < 8:
                        # seq-1 vP copies, 2 per slot over pairs 4-5 (DVE)
                        base = (p - 4) * 16 + 2 * f
                        emit_vP_half(1, base)
                        emit_vP_half(1, base + 1)
                        emit_vP_ones(1, (p - 4) * 8 + f)
                    if 1 <= p <= 6 and f < NQT:
                        emit_q_tr(p + 1, f)
                    if f >= 1:
                        emit_pv(p - 1, f - 1)
                        if p == 4:
                            emit_out_dma(0, f - 1)
            else:
                # last pair: compress PV(p-1) into early slots and start
                # PV(p) as soon as its enabling fill (and diag mask) lands
                for f in range(NFILL):
                    emit_fill(p, f)
                    if f == 0:
                        emit_pv(p - 1, 0)
                    elif f <= 3:
                        emit_pv(p - 1, 2 * f - 1)
                        emit_pv(p - 1, 2 * f)
                    elif f == 4:
                        emit_pv(p - 1, 7)
                    # PV(p, qt) ready after fill 5 (qt0), 6 (qt1,2),
                    # 7 (qt3..5), 8 (qt6,7)
                    if f == 5:
                        emit_pv(p, 0)
                        emit_out_dma(1, 0)
                    elif f == 6:
                        emit_pv(p, 1)
                        emit_out_dma(1, 1)
                        emit_pv(p, 2)
                        emit_out_dma(1, 2)
                    elif f == 7:
                        emit_pv(p, 3)
                        emit_out_dma(1, 3)
                        emit_pv(p, 4)
                        emit_out_dma(1, 4)
                    elif f == 8:
                        emit_pv(p, 5)
                        emit_out_dma(1, 5)
                for qt in range(6, NQT):
                    emit_pv(p, qt, pool=scp)
                    emit_out_dma(1, qt)
            if p >= 2:
                del es[p - 2]

    nc.compile()
    return nc


def _get_program(bt: np.ndarray):
    key = bt.tobytes()
    if key not in _CACHE:
        _CACHE[key] = _build_program(bt)
    return _CACHE[key]


def kernel(q, k_cache, v_cache, cu_seqlens_q, cu_seqlens_k, block_tables,
           _want_trace=False):
    from concourse import bass_utils

    q = np.ascontiguousarray(np.asarray(q, dtype=np.float32))
    k_cache = np.ascontiguousarray(np.asarray(k_cache, dtype=np.float32))
    v_cache = np.ascontiguousarray(np.asarray(v_cache, dtype=np.float32))
    bt = np.asarray(block_tables, dtype=np.int32)

    assert q.shape == (NUM_SEQS * LQ, NUM_HEADS, HEAD_DIM)
    assert k_cache.shape == (NUM_SEQS * NBLK, BLOCK_SIZE, NUM_KV_HEADS,
                             HEAD_DIM)
    assert v_cache.shape == k_cache.shape
    assert bt.shape == (NUM_SEQS, NBLK)
    assert bt.min() >= 0

    nc = _get_program(bt)

    in_maps = []
    for core in range(NUM_KV_HEADS):
        in_maps.append({
            "q": np.ascontiguousarray(
                q[:, core * GROUP:(core + 1) * GROUP, :]),
            "k": np.ascontiguousarray(
                k_cache[:, :, core, :]).reshape(NUM_SEQS * LK, HEAD_DIM),
            "v": np.ascontiguousarray(
                v_cache[:, :, core, :]).reshape(NUM_SEQS * LK, HEAD_DIM),
        })

    res = bass_utils.run_bass_kernel_spmd(
        nc, in_maps, core_ids=list(range(NUM_KV_HEADS)),
        trace=_want_trace,
        **({"trace_cores": list(range(NUM_KV_HEADS)), "stitch_traces": True}
           if _want_trace else {}),
    )

    out = np.empty((NUM_SEQS * LQ, NUM_HEADS, HEAD_DIM), dtype=np.float32)
    for core in range(NUM_KV_HEADS):
        out[:, core * GROUP:(core + 1) * GROUP, :] = res.results[core]["out"]

    if _want_trace:
        return out, res
    return out


# revision 7
# speedup vs baseline: 1.6862x; 1.0008x over previous
"""Paged prefill attention (sparse_attention) on 8 Trainium2 NeuronCores, v2.

Problem (hardcoded, mirrors the reference):
  q:        [2048, 32, 128] f32   (2 seqs x 1024 query tokens, 32 heads)
  k_cache:  [64, 64, 8, 128] f32  (64 physical blocks x 64 tokens x 8 kv heads)
  v_cache:  [64, 64, 8, 128] f32
  block_tables: [2, 32] int32 permutation of the 64 physical blocks
  out:      [2048, 32, 128] f32

Sharding: tensor-parallel by kv head. Core h gets kv head h plus its 4
query heads (GQA group 4), both full sequences.

v2 design vs the 250us baseline (which serialized 256 HWDGE DMA
descriptors at 625ns each):
  - K and V are loaded in PHYSICAL order with ONE dma_start each
    (partition = token%128 via a strided access pattern); the block-table
    permutation happens on-chip: K via per-half-block (64-partition) PE
    transposes into kT, V via partition-shifted DVE copies into vP.
  - Q: one DMA per (seq, head) = 8 total, [tok%128, qt, d] staging.
  - Output staged per seq as [q, 4*128] and written with one DMA per
    (seq, qtile) = 16 total.
  - Scores are computed in packed form: per (seq,head) pair the 16
    causally-clipped kv-chunk score panels (12800 columns total) are
    packed back-to-back into [128, 1536] PSUM fills -> 9 exp activation
    instructions per pair instead of 16 (ACT is the critical engine:
    0.833ns/col + ~185ns/instruction).
  - Software pipelining: fills (QK+exp) of pair p interleave with PV
    matmuls of pair p-1, keeping PE and ACT both ~100% busy; K/V/Q loads
    for seq 1 are sprinkled into early pair slots.
"""

import numpy as np

NUM_SEQS = 2
LQ = 1024
HIST = 1024
LK = LQ + HIST
NUM_HEADS = 32
NUM_KV_HEADS = 8
GROUP = NUM_HEADS // NUM_KV_HEADS  # 4 q heads per kv head / core
HEAD_DIM = 128
BLOCK_SIZE = 64
NBLK = LK // BLOCK_SIZE         # 32 logical blocks (64-token halves) per seq
NCH = LK // 128                 # 16 128-token kv chunks per sequence
NQT = LQ // 128                 # 8 128-token q tiles per sequence
SCALE = 1.0 / float(np.sqrt(HEAD_DIM))

# causal clipping per kv chunk kt: q columns [Q_LO[kt], 1024) are needed
Q_LO = [0] * 9 + [128 * i for i in range(1, 8)]
WIDTH = [LQ - lo for lo in Q_LO]
OFF = np.concatenate([[0], np.cumsum(WIDTH)]).tolist()  # packed col offsets
TOTAL_COLS = OFF[NCH]  # 12800
FILL = 1536            # packed columns per PSUM fill (3 banks)
NFILL = -(-TOTAL_COLS // FILL)  # 9

PAIRS = [(s, h) for s in range(NUM_SEQS) for h in range(GROUP)]

_CACHE = {}


def _plan_fills():
    """Static fill plan: for each fill, the packed window and the QK matmul
    segments (kt, packed_a, packed_b), split at chunk boundaries and at the
    512-col PSUM bank grid within the fill; plus which diag chunks' masks
    land wholly inside this fill."""
    fills = []
    for f in range(NFILL):
        lo, hi = f * FILL, min((f + 1) * FILL, TOTAL_COLS)
        segs = []
        for kt in range(NCH):
            a = max(OFF[kt], lo)
            b = min(OFF[kt + 1], hi)
            while a < b:
                # split at 512-grid relative to fill start
                nxt = lo + ((a - lo) // 512 + 1) * 512
                e = min(b, nxt)
                segs.append((kt, a, e))
                a = e
        diags = [kt for kt in range(NCH // 2, NCH)
                 if lo <= OFF[kt] and OFF[kt] + 128 <= hi]
        fills.append((lo, hi, segs, diags))
    return fills


FILLS = _plan_fills()


def _build_program(bt: np.ndarray):
    from contextlib import ExitStack

    import concourse.mybir as mybir
    import concourse.tile as tile
    from concourse import bacc
    from concourse.masks import make_identity

    f32 = mybir.dt.float32
    f16 = mybir.dt.float16

    nc = bacc.Bacc()
    q_d = nc.dram_tensor("q", [NUM_SEQS * LQ, GROUP, HEAD_DIM], f32,
                         kind="ExternalInput")
    k_d = nc.dram_tensor("k", [NUM_SEQS * LK, HEAD_DIM], f32,
                         kind="ExternalInput")
    v_d = nc.dram_tensor("v", [NUM_SEQS * LK, HEAD_DIM], f32,
                         kind="ExternalInput")
    o_d = nc.dram_tensor("out", [NUM_SEQS * LQ, GROUP, HEAD_DIM], f32,
                         kind="ExternalOutput")

    with tile.TileContext(nc) as tc, ExitStack() as ctx:
        consts = ctx.enter_context(tc.tile_pool(name="consts", bufs=1))
        kvstage = ctx.enter_context(tc.tile_pool(name="kvstage", bufs=1))
        qpool = ctx.enter_context(tc.tile_pool(name="qpool", bufs=4))
        persist = ctx.enter_context(tc.tile_pool(name="persist", bufs=1))
        espool = ctx.enter_context(tc.tile_pool(name="espool", bufs=3))
        opool = ctx.enter_context(tc.tile_pool(name="opool", bufs=2))
        small = ctx.enter_context(tc.tile_pool(name="small", bufs=8))
        scp = ctx.enter_context(tc.tile_pool(name="scp", bufs=2, space="PSUM"))
        ps1 = ctx.enter_context(tc.tile_pool(name="ps1", bufs=2, space="PSUM"))

        ident = consts.tile([128, 128], f32, tag="ident")
        make_identity(nc, ident[:, :])

        # persistent per-core data: kT [d, seq, chunk, tok], vP with ones col,
        # qT [d, seq, head, qcol]
        kT = persist.tile([128, NUM_SEQS, NCH, 128], f16, tag="kT")
        vP = persist.tile([128, NUM_SEQS * NCH, 129], f16, tag="vP")
        qT = persist.tile([128, NUM_SEQS, GROUP, LQ], f16, tag="qT")

        kst = kvstage.tile([128, NUM_SEQS * NCH, 128], f32, tag="kst")
        vst = kvstage.tile([128, NUM_SEQS * NCH, 128], f32, tag="vst")

        qstg = {}   # pair -> staging tile
        ostg = {}   # seq -> output staging tile
        es = {}     # pair -> packed exp-scores tile

        def kv_in_ap(t_d, lo_tok, n_tok):
            # DRAM rows [lo_tok, lo_tok+n_tok) enumerated (p, c, d),
            # row = c*128 + p
            return t_d[lo_tok:lo_tok + n_tok, :].rearrange(
                "(c p) d -> p c d", p=128)

        def emit_kv_dma(t_d, st, lo_tok, n_tok):
            c0 = lo_tok // 128
            nc.sync.dma_start(out=st[:, c0:c0 + n_tok // 128, :],
                              in_=kv_in_ap(t_d, lo_tok, n_tok))

        def emit_q_dma(p):
            s, h = PAIRS[p]
            qstg[p] = qpool.tile([128, NQT, 128], f32, tag="qstg",
                                 name=f"qstg{p}")
            in_ap = q_d[s * LQ:(s + 1) * LQ, h, :].rearrange(
                "(c p) d -> p c d", p=128)
            nc.sync.dma_start(out=qstg[p][:, :, :], in_=in_ap)

        def emit_kT_chunk(s, c):
            # logical chunk c of seq s: each 64-token half-block transposed
            # into its own PSUM tile (device rejects two transpose groups
            # sharing one PSUM tile)
            for h2 in range(2):
                b = int(bt[s, 2 * c + h2])
                pb = (b % 2) * 64
                pt = ps1.tile([128, 129], f32, tag="ps1",
                              name=f"tpk{s}_{c}_{h2}")
                nc.tensor.transpose(
                    pt[:, 0:64],
                    kst[pb:pb + 64, b // 2, :],
                    ident[pb:pb + 64, pb:pb + 64])
                nc.vector.tensor_copy(
                    kT[:, s, c, h2 * 64:(h2 + 1) * 64], pt[:, 0:64])

        def emit_vP_half(s, j):
            b = int(bt[s, j])
            c, h2 = j // 2, j % 2
            nc.vector.tensor_copy(
                vP[h2 * 64:(h2 + 1) * 64, s * NCH + c, 0:128],
                vst[(b % 2) * 64:(b % 2) * 64 + 64, b // 2, :])

        def emit_vP_ones(s, c):
            nc.vector.memset(vP[:, s * NCH + c, 128:129], 1.0)

        def emit_q_tr(p, qt):
            s, h = PAIRS[p]
            pt = ps1.tile([128, 129], f32, tag="ps1", name=f"tpq{p}_{qt}")
            nc.tensor.transpose(pt[:, 0:128], qstg[p][:, qt, :], ident[:, :])
            nc.vector.tensor_copy(
                qT[:, s, h, qt * 128:(qt + 1) * 128], pt[:, 0:128])

        def emit_fill(p, f):
            s, h = PAIRS[p]
            lo, hi, segs, diags = FILLS[f]
            sc = scp.tile([128, FILL], f32, tag="sc", name=f"sc{p}_{f}")
            for kt, a, b in segs:
                qcol = a - OFF[kt] + Q_LO[kt]
                nc.tensor.matmul(
                    sc[:, a - lo:b - lo],
                    kT[:, s, kt, :],
                    qT[:, s, h, qcol:qcol + (b - a)],
                    start=True, stop=True)
            nc.scalar.activation(
                es[p][:, lo:hi], sc[:, 0:hi - lo],
                mybir.ActivationFunctionType.Exp, scale=SCALE)
            for kt in diags:
                # zero the strictly-lower triangle of the diag panel
                dc = OFF[kt]
                nc.gpsimd.affine_select(
                    out=es[p][:, dc:dc + 128], in_=es[p][:, dc:dc + 128],
                    compare_op=mybir.AluOpType.is_ge, fill=0.0,
                    base=0, pattern=[[1, 128]], channel_multiplier=-1)

        def emit_pv(p, qt, pool=None):
            s, h = PAIRS[p]
            nch_q = NCH // 2 + 1 + qt   # kv chunks 0 .. 8+qt
            if pool is None:
                po = ps1.tile([128, 129], f32, tag="ps1", name=f"po{p}_{qt}")
            else:
                po = pool.tile([128, 129], f32, tag="sc", name=f"po{p}_{qt}")
            for c in range(nch_q):
                col = OFF[c] + qt * 128 - Q_LO[c]
                nc.tensor.matmul(
                    po[:, :],
                    es[p][:, col:col + 128],
                    vP[:, s * NCH + c, :],
                    start=(c == 0), stop=(c == nch_q - 1))
            rc = small.tile([128, 1], f32, tag="rc", name=f"rc{p}_{qt}")
            nc.vector.reciprocal(rc[:, :], po[:, 128:129])
            nc.vector.tensor_scalar_mul(
                ostg[s][:, qt, h * 128:(h + 1) * 128], po[:, 0:128], rc[:, :])

        def emit_out_dma(s, qt):
            nc.sync.dma_start(
                out=o_d[s * LQ + qt * 128:s * LQ + (qt + 1) * 128, :, :],
                in_=ostg[s][:, qt, :])

        # per-fill kT-chunk prerequisites (cumulative)
        chunks_needed = [max(kt for kt, _, _ in FILLS[f][2]) + 1
                         for f in range(NFILL)]

        # ---------------- load phase ----------------
        # DMA issue order gates the (shared, serial) DMA engines: K first
        # (kT feeds the first QK fills), then q0/q1, then V (vP is only
        # needed once PV of pair 0 starts, one pair-time later).
        emit_q_dma(0)
        for quarter in range(4):
            emit_kv_dma(k_d, kst, quarter * 1024, 1024)
        emit_q_dma(1)
        emit_kv_dma(v_d, vst, 0, LK)
        emit_kv_dma(v_d, vst, LK, LK)

        # ---------------- pair 0: fills + just-in-time transposes -------
        es[0] = espool.tile([128, TOTAL_COLS], f16, tag="es", name="es0")
        for qt in range(NQT):
            emit_q_tr(0, qt)
        done_chunks = 0
        for f in range(NFILL):
            while done_chunks < chunks_needed[f]:
                emit_kT_chunk(0, done_chunks)
                done_chunks += 1
            emit_fill(0, f)
            if 4 <= f <= 7:
                emit_q_tr(1, f - 4)
                emit_q_tr(1, f)
        while done_chunks < NCH:
            emit_kT_chunk(0, done_chunks)
            done_chunks += 1
        # vP for seq 0 (DVE-only block; V DMA has landed by now)
        for j in range(NBLK):
            emit_vP_half(0, j)
        for c in range(NCH):
            emit_vP_ones(0, c)

        # ---------------- steady pairs 1..7 ----------------
        LAST = len(PAIRS) - 1
        for p in range(1, len(PAIRS)):
            s, h = PAIRS[p]
            if p == 1:
                ostg[0] = opool.tile([128, NQT, GROUP * 128], f32,
                                     tag="ostg", name="ostg0")
            if p == 5:
                ostg[1] = opool.tile([128, NQT, GROUP * 128], f32,
                                     tag="ostg", name="ostg1")
            if 1 <= p <= 6:
                emit_q_dma(p + 1)
            es[p] = espool.tile([128, TOTAL_COLS], f16, tag="es",
                                name=f"es{p}")
            if p < LAST:
                # fills of pair p interleave PV batches of pair p-1
                for f in range(NFILL):
                    emit_fill(p, f)
                    if p in (1, 2) and f >= 3:
                        # seq-1 kT chunks: 6 per pair over pairs 1-2 (f3..f8)
                        emit_kT_chunk(1, (p - 1) * 6 + (f - 3))
                    if p == 3 and 5 <= f <= 8:
                        emit_kT_chunk(1, 12 + (f - 5))
                    if p in (4, 5) and f < 8:
                        # seq-1 vP copies, 2 per slot over pairs 4-5 (DVE)
                        base = (p - 4) * 16 + 2 * f
                        emit_vP_half(1, base)
                        emit_vP_half(1, base + 1)
                        emit_vP_ones(1, (p - 4) * 8 + f)
                    if 1 <= p <= 6 and f < NQT:
                        emit_q_tr(p + 1, f)
                    if f >= 1:
                        emit_pv(p - 1, f - 1)
                        if p == 4:
                            emit_out_dma(0, f - 1)
            else:
                # last pair: compress PV(p-1) into early slots and start
                # PV(p) as soon as its enabling fill (and diag mask) lands
                for f in range(NFILL):
                    emit_fill(p, f)
                    if f == 0:
                        emit_pv(p - 1, 0)
                    elif f <= 3:
                        emit_pv(p - 1, 2 * f - 1)
                        emit_pv(p - 1, 2 * f)
                    elif f == 4:
                        emit_pv(p - 1, 7)
                    # PV(p, qt) ready after fill 5 (qt0), 6 (qt1,2),
                    # 7 (qt3..5), 8 (qt6,7)
                    if f == 5:
                        emit_pv(p, 0)
                        emit_out_dma(1, 0)
                    elif f == 6:
                        emit_pv(p, 1)
                        emit_out_dma(1, 1)
                        emit_pv(p, 2)
                        emit_out_dma(1, 2)
                    elif f == 7:
                        emit_pv(p, 3)
                        emit_out_dma(1, 3)
                        emit_pv(p, 4)
                        emit_out_dma(1, 4)
                    elif f == 8:
                        emit_pv(p, 5)
                        emit_out_dma(1, 5)
                for qt in range(6, NQT):
                    emit_pv(p, qt, pool=scp)
                    emit_out_dma(1, qt)
            if p >= 2:
                del es[p - 2]

    nc.compile()
    return nc


def _get_program(bt: np.ndarray):
    key = bt.tobytes()
    if key not in _CACHE:
        _CACHE[key] = _build_program(bt)
    return _CACHE[key]


def kernel(q, k_cache, v_cache, cu_seqlens_q, cu_seqlens_k, block_tables,
           _want_trace=False):
    from concourse import bass_utils

    q = np.ascontiguousarray(np.asarray(q, dtype=np.float32))
    k_cache = np.ascontiguousarray(np.asarray(k_cache, dtype=np.float32))
    v_cache = np.ascontiguousarray(np.asarray(v_cache, dtype=np.float32))
    bt = np.asarray(block_tables, dtype=np.int32)

    assert q.shape == (NUM_SEQS * LQ, NUM_HEADS, HEAD_DIM)
    assert k_cache.shape == (NUM_SEQS * NBLK, BLOCK_SIZE, NUM_KV_HEADS,
                             HEAD_DIM)
    assert v_cache.shape == k_cache.shape
    assert bt.shape == (NUM_SEQS, NBLK)
    assert bt.min() >= 0

    nc = _get_program(bt)

    in_maps = []
    for core in range(NUM_KV_HEADS):
        in_maps.append({
            "q": np.ascontiguousarray(
                q[:, core * GROUP:(core + 1) * GROUP, :]),
            "k": np.ascontiguousarray(
                k_cache[:, :, core, :]).reshape(NUM_SEQS * LK, HEAD_DIM),
            "v": np.ascontiguousarray(
                v_cache[:, :, core, :]).reshape(NUM_SEQS * LK, HEAD_DIM),
        })

    res = bass_utils.run_bass_kernel_spmd(
        nc, in_maps, core_ids=list(range(NUM_KV_HEADS)),
        trace=_want_trace,
        **({"trace_cores": list(range(NUM_KV_HEADS)), "stitch_traces": True}
           if _want_trace else {}),
    )

    out = np.empty((NUM_SEQS * LQ, NUM_HEADS, HEAD_DIM), dtype=np.float32)
    for core in range(NUM_KV_HEADS):
        out[:, core * GROUP:(core + 1) * GROUP, :] = res.results[core]["out"]

    if _want_trace:
        return out, res
    return out


# revision 9
# speedup vs baseline: 1.7231x; 1.0219x over previous
"""Paged prefill attention (sparse_attention) on 8 Trainium2 NeuronCores, v2.

Problem (hardcoded, mirrors the reference):
  q:        [2048, 32, 128] f32   (2 seqs x 1024 query tokens, 32 heads)
  k_cache:  [64, 64, 8, 128] f32  (64 physical blocks x 64 tokens x 8 kv heads)
  v_cache:  [64, 64, 8, 128] f32
  block_tables: [2, 32] int32 permutation of the 64 physical blocks
  out:      [2048, 32, 128] f32

Sharding: tensor-parallel by kv head. Core h gets kv head h plus its 4
query heads (GQA group 4), both full sequences.

v2 design vs the 250us baseline (which serialized 256 HWDGE DMA
descriptors at 625ns each):
  - K and V are loaded in PHYSICAL order with ONE dma_start each
    (partition = token%128 via a strided access pattern); the block-table
    permutation happens on-chip: K via per-half-block (64-partition) PE
    transposes into kT, V via partition-shifted DVE copies into vP.
  - Q: one DMA per (seq, head) = 8 total, [tok%128, qt, d] staging.
  - Output staged per seq as [q, 4*128] and written with one DMA per
    (seq, qtile) = 16 total.
  - Scores are computed in packed form: per (seq,head) pair the 16
    causally-clipped kv-chunk score panels (12800 columns total) are
    packed back-to-back into [128, 1536] PSUM fills -> 9 exp activation
    instructions per pair instead of 16 (ACT is the critical engine:
    0.833ns/col + ~185ns/instruction).
  - Software pipelining: fills (QK+exp) of pair p interleave with PV
    matmuls of pair p-1, keeping PE and ACT both ~100% busy; K/V/Q loads
    for seq 1 are sprinkled into early pair slots.
"""

import numpy as np

NUM_SEQS = 2
LQ = 1024
HIST = 1024
LK = LQ + HIST
NUM_HEADS = 32
NUM_KV_HEADS = 8
GROUP = NUM_HEADS // NUM_KV_HEADS  # 4 q heads per kv head / core
HEAD_DIM = 128
BLOCK_SIZE = 64
NBLK = LK // BLOCK_SIZE         # 32 logical blocks (64-token halves) per seq
NCH = LK // 128                 # 16 128-token kv chunks per sequence
NQT = LQ // 128                 # 8 128-token q tiles per sequence
SCALE = 1.0 / float(np.sqrt(HEAD_DIM))

# causal clipping per kv chunk kt: q columns [Q_LO[kt], 1024) are needed
Q_LO = [0] * 9 + [128 * i for i in range(1, 8)]
WIDTH = [LQ - lo for lo in Q_LO]
OFF = np.concatenate([[0], np.cumsum(WIDTH)]).tolist()  # packed col offsets
TOTAL_COLS = OFF[NCH]  # 12800
FILL = 1536            # packed columns per PSUM fill (3 banks)
NFILL = -(-TOTAL_COLS // FILL)  # 9

PAIRS = [(s, h) for s in range(NUM_SEQS) for h in range(GROUP)]

_CACHE = {}


def _plan_fills():
    """Static fill plan: for each fill, the packed window and the QK matmul
    segments (kt, packed_a, packed_b), split at chunk boundaries and at the
    512-col PSUM bank grid within the fill; plus which diag chunks' masks
    land wholly inside this fill."""
    fills = []
    for f in range(NFILL):
        lo, hi = f * FILL, min((f + 1) * FILL, TOTAL_COLS)
        segs = []
        for kt in range(NCH):
            a = max(OFF[kt], lo)
            b = min(OFF[kt + 1], hi)
            while a < b:
                # split at 512-grid relative to fill start
                nxt = lo + ((a - lo) // 512 + 1) * 512
                e = min(b, nxt)
                segs.append((kt, a, e))
                a = e
        diags = [kt for kt in range(NCH // 2, NCH)
                 if lo <= OFF[kt] and OFF[kt] + 128 <= hi]
        fills.append((lo, hi, segs, diags))
    return fills


FILLS = _plan_fills()


def _build_program(bt: np.ndarray):
    from contextlib import ExitStack

    import concourse.mybir as mybir
    import concourse.tile as tile
    from concourse import bacc
    from concourse.masks import make_identity

    f32 = mybir.dt.float32
    f16 = mybir.dt.float16

    nc = bacc.Bacc()
    q_d = nc.dram_tensor("q", [NUM_SEQS * LQ, GROUP, HEAD_DIM], f32,
                         kind="ExternalInput")
    k_d = nc.dram_tensor("k", [NUM_SEQS * LK, HEAD_DIM], f32,
                         kind="ExternalInput")
    v_d = nc.dram_tensor("v", [NUM_SEQS * LK, HEAD_DIM], f32,
                         kind="ExternalInput")
    o_d = nc.dram_tensor("out", [NUM_SEQS * LQ, GROUP, HEAD_DIM], f32,
                         kind="ExternalOutput")

    with tile.TileContext(nc) as tc, ExitStack() as ctx:
        consts = ctx.enter_context(tc.tile_pool(name="consts", bufs=1))
        kvstage = ctx.enter_context(tc.tile_pool(name="kvstage", bufs=1))
        qpool = ctx.enter_context(tc.tile_pool(name="qpool", bufs=4))
        persist = ctx.enter_context(tc.tile_pool(name="persist", bufs=1))
        espool = ctx.enter_context(tc.tile_pool(name="espool", bufs=3))
        opool = ctx.enter_context(tc.tile_pool(name="opool", bufs=2))
        small = ctx.enter_context(tc.tile_pool(name="small", bufs=8))
        scp = ctx.enter_context(tc.tile_pool(name="scp", bufs=2, space="PSUM"))
        ps1 = ctx.enter_context(tc.tile_pool(name="ps1", bufs=2, space="PSUM"))

        ident = consts.tile([128, 128], f32, tag="ident")
        make_identity(nc, ident[:, :])

        # persistent per-core data: kT [d, seq, chunk, tok], vP with ones col,
        # qT [d, seq, head, qcol]
        kT = persist.tile([128, NUM_SEQS, NCH, 128], f16, tag="kT")
        vP = persist.tile([128, NUM_SEQS * NCH, 129], f16, tag="vP")
        qT = persist.tile([128, NUM_SEQS, GROUP, LQ], f16, tag="qT")

        kst = kvstage.tile([128, NUM_SEQS * NCH, 128], f32, tag="kst")
        vst = kvstage.tile([128, NUM_SEQS * NCH, 128], f32, tag="vst")

        qstg = {}   # pair -> staging tile
        ostg = {}   # seq -> output staging tile
        es = {}     # pair -> packed exp-scores tile

        def kv_in_ap(t_d, lo_tok, n_tok):
            # DRAM rows [lo_tok, lo_tok+n_tok) enumerated (p, c, d),
            # row = c*128 + p
            return t_d[lo_tok:lo_tok + n_tok, :].rearrange(
                "(c p) d -> p c d", p=128)

        def emit_kv_dma(t_d, st, lo_tok, n_tok):
            c0 = lo_tok // 128
            nc.sync.dma_start(out=st[:, c0:c0 + n_tok // 128, :],
                              in_=kv_in_ap(t_d, lo_tok, n_tok))

        def emit_q_dma(p):
            s, h = PAIRS[p]
            qstg[p] = qpool.tile([128, NQT, 128], f32, tag="qstg",
                                 name=f"qstg{p}")
            in_ap = q_d[s * LQ:(s + 1) * LQ, h, :].rearrange(
                "(c p) d -> p c d", p=128)
            nc.sync.dma_start(out=qstg[p][:, :, :], in_=in_ap)

        def emit_kT_chunk(s, c):
            # logical chunk c of seq s: each 64-token half-block transposed
            # into its own PSUM tile (device rejects two transpose groups
            # sharing one PSUM tile)
            for h2 in range(2):
                b = int(bt[s, 2 * c + h2])
                pb = (b % 2) * 64
                pt = ps1.tile([128, 129], f32, tag="ps1",
                              name=f"tpk{s}_{c}_{h2}")
                nc.tensor.transpose(
                    pt[:, 0:64],
                    kst[pb:pb + 64, b // 2, :],
                    ident[pb:pb + 64, pb:pb + 64])
                nc.vector.tensor_copy(
                    kT[:, s, c, h2 * 64:(h2 + 1) * 64], pt[:, 0:64])

        def emit_vP_half(s, j):
            b = int(bt[s, j])
            c, h2 = j // 2, j % 2
            nc.vector.tensor_copy(
                vP[h2 * 64:(h2 + 1) * 64, s * NCH + c, 0:128],
                vst[(b % 2) * 64:(b % 2) * 64 + 64, b // 2, :])

        def emit_vP_ones(s, c):
            nc.vector.memset(vP[:, s * NCH + c, 128:129], 1.0)

        def emit_q_tr(p, qt):
            s, h = PAIRS[p]
            pt = ps1.tile([128, 129], f32, tag="ps1", name=f"tpq{p}_{qt}")
            nc.tensor.transpose(pt[:, 0:128], qstg[p][:, qt, :], ident[:, :])
            nc.vector.tensor_copy(
                qT[:, s, h, qt * 128:(qt + 1) * 128], pt[:, 0:128])

        def emit_fill(p, f):
            s, h = PAIRS[p]
            lo, hi, segs, diags = FILLS[f]
            sc = scp.tile([128, FILL], f32, tag="sc", name=f"sc{p}_{f}")
            for kt, a, b in segs:
                qcol = a - OFF[kt] + Q_LO[kt]
                nc.tensor.matmul(
                    sc[:, a - lo:b - lo],
                    kT[:, s, kt, :],
                    qT[:, s, h, qcol:qcol + (b - a)],
                    start=True, stop=True)
            nc.scalar.activation(
                es[p][:, lo:hi], sc[:, 0:hi - lo],
                mybir.ActivationFunctionType.Exp, scale=SCALE)
            for kt in diags:
                # zero the strictly-lower triangle of the diag panel
                dc = OFF[kt]
                nc.gpsimd.affine_select(
                    out=es[p][:, dc:dc + 128], in_=es[p][:, dc:dc + 128],
                    compare_op=mybir.AluOpType.is_ge, fill=0.0,
                    base=0, pattern=[[1, 128]], channel_multiplier=-1)

        def emit_pv(p, qt, pool=None):
            s, h = PAIRS[p]
            nch_q = NCH // 2 + 1 + qt   # kv chunks 0 .. 8+qt
            if pool is None:
                po = ps1.tile([128, 129], f32, tag="ps1", name=f"po{p}_{qt}")
            else:
                po = pool.tile([128, 129], f32, tag="sc", name=f"po{p}_{qt}")
            for c in range(nch_q):
                col = OFF[c] + qt * 128 - Q_LO[c]
                nc.tensor.matmul(
                    po[:, :],
                    es[p][:, col:col + 128],
                    vP[:, s * NCH + c, :],
                    start=(c == 0), stop=(c == nch_q - 1))
            rc = small.tile([128, 1], f32, tag="rc", name=f"rc{p}_{qt}")
            nc.vector.reciprocal(rc[:, :], po[:, 128:129])
            nc.vector.tensor_scalar_mul(
                ostg[s][:, qt, h * 128:(h + 1) * 128], po[:, 0:128], rc[:, :])

        def emit_out_dma(s, qt):
            nc.sync.dma_start(
                out=o_d[s * LQ + qt * 128:s * LQ + (qt + 1) * 128, :, :],
                in_=ostg[s][:, qt, :])

        # per-fill kT-chunk prerequisites (cumulative)
        chunks_needed = [max(kt for kt, _, _ in FILLS[f][2]) + 1
                         for f in range(NFILL)]

        # ---------------- load phase ----------------
        # DMA issue order gates the (shared, serial) DMA engines: K first
        # (kT feeds the first QK fills), then q0/q1, then V (vP is only
        # needed once PV of pair 0 starts, one pair-time later).
        emit_q_dma(0)
        # issue K quarters in order of earliest logical use by seq 0 so the
        # first fills' chunks land first (bt is baked into this program)
        def _qprio(qu):
            uses = [j // 2 for j in range(NBLK) if int(bt[0, j]) // 16 == qu]
            return min(uses) if uses else 99
        for quarter in sorted(range(4), key=_qprio):
            emit_kv_dma(k_d, kst, quarter * 1024, 1024)
        emit_q_dma(1)
        emit_kv_dma(v_d, vst, 0, LK)
        emit_kv_dma(v_d, vst, LK, LK)

        # ---------------- pair 0: fills + just-in-time transposes -------
        es[0] = espool.tile([128, TOTAL_COLS], f16, tag="es", name="es0")
        for qt in range(NQT):
            emit_q_tr(0, qt)
        done_chunks = 0
        for f in range(NFILL):
            while done_chunks < chunks_needed[f]:
                emit_kT_chunk(0, done_chunks)
                done_chunks += 1
            emit_fill(0, f)
            if 4 <= f <= 7:
                emit_q_tr(1, f - 4)
                emit_q_tr(1, f)
        while done_chunks < NCH:
            emit_kT_chunk(0, done_chunks)
            done_chunks += 1
        # vP for seq 0 (DVE-only block; V DMA has landed by now)
        for j in range(NBLK):
            emit_vP_half(0, j)
        for c in range(NCH):
            emit_vP_ones(0, c)

        # ---------------- steady pairs 1..7 ----------------
        LAST = len(PAIRS) - 1
        for p in range(1, len(PAIRS)):
            s, h = PAIRS[p]
            if p == 1:
                ostg[0] = opool.tile([128, NQT, GROUP * 128], f32,
                                     tag="ostg", name="ostg0")
            if p == 5:
                ostg[1] = opool.tile([128, NQT, GROUP * 128], f32,
                                     tag="ostg", name="ostg1")
            if 1 <= p <= 6:
                emit_q_dma(p + 1)
            es[p] = espool.tile([128, TOTAL_COLS], f16, tag="es",
                                name=f"es{p}")
            if p < LAST:
                # fills of pair p interleave PV batches of pair p-1
                for f in range(NFILL):
                    emit_fill(p, f)
                    if p in (1, 2) and f >= 3:
                        # seq-1 kT chunks: 6 per pair over pairs 1-2 (f3..f8)
                        emit_kT_chunk(1, (p - 1) * 6 + (f - 3))
                    if p == 3 and 5 <= f <= 8:
                        emit_kT_chunk(1, 12 + (f - 5))
                    if p in (4, 5) and f < 8:
                        # seq-1 vP copies, 2 per slot over pairs 4-5 (DVE)
                        base = (p - 4) * 16 + 2 * f
                        emit_vP_half(1, base)
                        emit_vP_half(1, base + 1)
                        emit_vP_ones(1, (p - 4) * 8 + f)
                    if 1 <= p <= 6 and f < NQT:
                        emit_q_tr(p + 1, f)
                    if f >= 1:
                        emit_pv(p - 1, f - 1)
                        if p == 5:
                            # seq-0 outputs: data has been ready since pair 4,
                            # so the DMA waits never block the SP sequencer
                            emit_out_dma(0, f - 1)
            else:
                # last pair: compress PV(p-1) into early slots and start
                # PV(p) as soon as its enabling fill (and diag mask) lands
                for f in range(NFILL):
                    emit_fill(p, f)
                    if f == 0:
                        emit_pv(p - 1, 0)
                    elif f <= 3:
                        emit_pv(p - 1, 2 * f - 1)
                        emit_pv(p - 1, 2 * f)
                    elif f == 4:
                        emit_pv(p - 1, 7)
                        # heads 0-2 of (seq1, qt7) are final after pair 6:
                        # write them now so the closing DMA only carries
                        # head 3's slice
                        nc.sync.dma_start(
                            out=o_d[LQ + 7 * 128:LQ + 8 * 128, 0:3, :],
                            in_=ostg[1][:, 7, 0:384])
                    # PV(p, qt) ready after fill 5 (qt0), 6 (qt1,2),
                    # 7 (qt3..5), 8 (qt6,7)
                    if f == 5:
                        emit_pv(p, 0)
                        emit_out_dma(1, 0)
                    elif f == 6:
                        emit_pv(p, 1)
                        emit_out_dma(1, 1)
                        emit_pv(p, 2)
                        emit_out_dma(1, 2)
                    elif f == 7:
                        emit_pv(p, 3)
                        emit_out_dma(1, 3)
                        emit_pv(p, 4)
                        emit_out_dma(1, 4)
                    elif f == 8:
                        emit_pv(p, 5)
                        emit_out_dma(1, 5)
                emit_pv(p, 6, pool=scp)
                emit_out_dma(1, 6)
                emit_pv(p, 7, pool=scp)
                nc.sync.dma_start(
                    out=o_d[LQ + 7 * 128:LQ + 8 * 128, 3, :],
                    in_=ostg[1][:, 7, 384:512])
            if p >= 2:
                del es[p - 2]

    nc.compile()
    return nc


def _get_program(bt: np.ndarray):
    key = bt.tobytes()
    if key not in _CACHE:
        _CACHE[key] = _build_program(bt)
    return _CACHE[key]


def kernel(q, k_cache, v_cache, cu_seqlens_q, cu_seqlens_k, block_tables,
           _want_trace=False):
    from concourse import bass_utils

    q = np.ascontiguousarray(np.asarray(q, dtype=np.float32))
    k_cache = np.ascontiguousarray(np.asarray(k_cache, dtype=np.float32))
    v_cache = np.ascontiguousarray(np.asarray(v_cache, dtype=np.float32))
    bt = np.asarray(block_tables, dtype=np.int32)

    assert q.shape == (NUM_SEQS * LQ, NUM_HEADS, HEAD_DIM)
    assert k_cache.shape == (NUM_SEQS * NBLK, BLOCK_SIZE, NUM_KV_HEADS,
                             HEAD_DIM)
    assert v_cache.shape == k_cache.shape
    assert bt.shape == (NUM_SEQS, NBLK)
    assert bt.min() >= 0

    nc = _get_program(bt)

    in_maps = []
    for core in range(NUM_KV_HEADS):
        in_maps.append({
            "q": np.ascontiguousarray(
                q[:, core * GROUP:(core + 1) * GROUP, :]),
            "k": np.ascontiguousarray(
                k_cache[:, :, core, :]).reshape(NUM_SEQS * LK, HEAD_DIM),
            "v": np.ascontiguousarray(
                v_cache[:, :, core, :]).reshape(NUM_SEQS * LK, HEAD_DIM),
        })

    res = bass_utils.run_bass_kernel_spmd(
        nc, in_maps, core_ids=list(range(NUM_KV_HEADS)),
        trace=_want_trace,
        **({"trace_cores": list(range(NUM_KV_HEADS)), "stitch_traces": True}
           if _want_trace else {}),
    )

    out = np.empty((NUM_SEQS * LQ, NUM_HEADS, HEAD_DIM), dtype=np.float32)
    for core in range(NUM_KV_HEADS):
        out[:, core * GROUP:(core + 1) * GROUP, :] = res.results[core]["out"]

    if _want_trace:
        return out, res
    return out


# revision 10
# speedup vs baseline: 1.7258x; 1.0016x over previous
"""Paged prefill attention (sparse_attention) on 8 Trainium2 NeuronCores, v2.

Problem (hardcoded, mirrors the reference):
  q:        [2048, 32, 128] f32   (2 seqs x 1024 query tokens, 32 heads)
  k_cache:  [64, 64, 8, 128] f32  (64 physical blocks x 64 tokens x 8 kv heads)
  v_cache:  [64, 64, 8, 128] f32
  block_tables: [2, 32] int32 permutation of the 64 physical blocks
  out:      [2048, 32, 128] f32

Sharding: tensor-parallel by kv head. Core h gets kv head h plus its 4
query heads (GQA group 4), both full sequences.

v2 design vs the 250us baseline (which serialized 256 HWDGE DMA
descriptors at 625ns each):
  - K and V are loaded in PHYSICAL order with ONE dma_start each
    (partition = token%128 via a strided access pattern); the block-table
    permutation happens on-chip: K via per-half-block (64-partition) PE
    transposes into kT, V via partition-shifted DVE copies into vP.
  - Q: one DMA per (seq, head) = 8 total, [tok%128, qt, d] staging.
  - Output staged per seq as [q, 4*128] and written with one DMA per
    (seq, qtile) = 16 total.
  - Scores are computed in packed form: per (seq,head) pair the 16
    causally-clipped kv-chunk score panels (12800 columns total) are
    packed back-to-back into [128, 1536] PSUM fills -> 9 exp activation
    instructions per pair instead of 16 (ACT is the critical engine:
    0.833ns/col + ~185ns/instruction).
  - Software pipelining: fills (QK+exp) of pair p interleave with PV
    matmuls of pair p-1, keeping PE and ACT both ~100% busy; K/V/Q loads
    for seq 1 are sprinkled into early pair slots.
"""

import numpy as np

NUM_SEQS = 2
LQ = 1024
HIST = 1024
LK = LQ + HIST
NUM_HEADS = 32
NUM_KV_HEADS = 8
GROUP = NUM_HEADS // NUM_KV_HEADS  # 4 q heads per kv head / core
HEAD_DIM = 128
BLOCK_SIZE = 64
NBLK = LK // BLOCK_SIZE         # 32 logical blocks (64-token halves) per seq
NCH = LK // 128                 # 16 128-token kv chunks per sequence
NQT = LQ // 128                 # 8 128-token q tiles per sequence
SCALE = 1.0 / float(np.sqrt(HEAD_DIM))

# causal clipping per kv chunk kt: q columns [Q_LO[kt], 1024) are needed
Q_LO = [0] * 9 + [128 * i for i in range(1, 8)]
WIDTH = [LQ - lo for lo in Q_LO]
OFF = np.concatenate([[0], np.cumsum(WIDTH)]).tolist()  # packed col offsets
TOTAL_COLS = OFF[NCH]  # 12800
FILL = 1536            # packed columns per PSUM fill (3 banks)
NFILL = -(-TOTAL_COLS // FILL)  # 9

PAIRS = [(s, h) for s in range(NUM_SEQS) for h in range(GROUP)]

_CACHE = {}


def _plan_fills():
    """Static fill plan: for each fill, the packed window and the QK matmul
    segments (kt, packed_a, packed_b), split at chunk boundaries and at the
    512-col PSUM bank grid within the fill; plus which diag chunks' masks
    land wholly inside this fill."""
    fills = []
    for f in range(NFILL):
        lo, hi = f * FILL, min((f + 1) * FILL, TOTAL_COLS)
        segs = []
        for kt in range(NCH):
            a = max(OFF[kt], lo)
            b = min(OFF[kt + 1], hi)
            while a < b:
                # split at 512-grid relative to fill start
                nxt = lo + ((a - lo) // 512 + 1) * 512
                e = min(b, nxt)
                segs.append((kt, a, e))
                a = e
        diags = [kt for kt in range(NCH // 2, NCH)
                 if lo <= OFF[kt] and OFF[kt] + 128 <= hi]
        fills.append((lo, hi, segs, diags))
    return fills


FILLS = _plan_fills()


def _build_program(bt: np.ndarray):
    from contextlib import ExitStack

    import concourse.mybir as mybir
    import concourse.tile as tile
    from concourse import bacc
    from concourse.masks import make_identity

    f32 = mybir.dt.float32
    f16 = mybir.dt.float16

    nc = bacc.Bacc()
    q_d = nc.dram_tensor("q", [NUM_SEQS * LQ, GROUP, HEAD_DIM], f32,
                         kind="ExternalInput")
    k_d = nc.dram_tensor("k", [NUM_SEQS * LK, HEAD_DIM], f32,
                         kind="ExternalInput")
    v_d = nc.dram_tensor("v", [NUM_SEQS * LK, HEAD_DIM], f32,
                         kind="ExternalInput")
    o_d = nc.dram_tensor("out", [NUM_SEQS * LQ, GROUP, HEAD_DIM], f32,
                         kind="ExternalOutput")

    with tile.TileContext(nc) as tc, ExitStack() as ctx:
        consts = ctx.enter_context(tc.tile_pool(name="consts", bufs=1))
        kvstage = ctx.enter_context(tc.tile_pool(name="kvstage", bufs=1))
        qpool = ctx.enter_context(tc.tile_pool(name="qpool", bufs=4))
        persist = ctx.enter_context(tc.tile_pool(name="persist", bufs=1))
        espool = ctx.enter_context(tc.tile_pool(name="espool", bufs=3))
        opool = ctx.enter_context(tc.tile_pool(name="opool", bufs=2))
        small = ctx.enter_context(tc.tile_pool(name="small", bufs=8))
        scp = ctx.enter_context(tc.tile_pool(name="scp", bufs=2, space="PSUM"))
        ps1 = ctx.enter_context(tc.tile_pool(name="ps1", bufs=2, space="PSUM"))

        ident = consts.tile([128, 128], f32, tag="ident")
        make_identity(nc, ident[:, :])

        # persistent per-core data: kT [d, seq, chunk, tok], vP with ones col,
        # qT [d, seq, head, qcol]
        kT = persist.tile([128, NUM_SEQS, NCH, 128], f16, tag="kT")
        vP = persist.tile([128, NUM_SEQS * NCH, 129], f16, tag="vP")
        qT = persist.tile([128, NUM_SEQS, GROUP, LQ], f16, tag="qT")

        kst = kvstage.tile([128, NUM_SEQS * NCH, 128], f32, tag="kst")
        vst = kvstage.tile([128, NUM_SEQS * NCH, 128], f32, tag="vst")

        qstg = {}   # pair -> staging tile
        ostg = {}   # seq -> output staging tile
        es = {}     # pair -> packed exp-scores tile

        def kv_in_ap(t_d, lo_tok, n_tok):
            # DRAM rows [lo_tok, lo_tok+n_tok) enumerated (p, c, d),
            # row = c*128 + p
            return t_d[lo_tok:lo_tok + n_tok, :].rearrange(
                "(c p) d -> p c d", p=128)

        def emit_kv_dma(t_d, st, lo_tok, n_tok):
            c0 = lo_tok // 128
            nc.sync.dma_start(out=st[:, c0:c0 + n_tok // 128, :],
                              in_=kv_in_ap(t_d, lo_tok, n_tok))

        def emit_q_dma(p, split=False):
            s, h = PAIRS[p]
            qstg[p] = qpool.tile([128, NQT, 128], f32, tag="qstg",
                                 name=f"qstg{p}")
            if split:
                # halve the first q DMA so the lead-in transposes start
                # as soon as the first four q tiles land
                for half in range(2):
                    in_ap = q_d[s * LQ + half * 512:s * LQ + (half + 1) * 512,
                                h, :].rearrange("(c p) d -> p c d", p=128)
                    nc.sync.dma_start(
                        out=qstg[p][:, half * 4:(half + 1) * 4, :], in_=in_ap)
            else:
                in_ap = q_d[s * LQ:(s + 1) * LQ, h, :].rearrange(
                    "(c p) d -> p c d", p=128)
                nc.sync.dma_start(out=qstg[p][:, :, :], in_=in_ap)

        def emit_kT_chunk(s, c):
            # logical chunk c of seq s: each 64-token half-block transposed
            # into its own PSUM tile (device rejects two transpose groups
            # sharing one PSUM tile)
            for h2 in range(2):
                b = int(bt[s, 2 * c + h2])
                pb = (b % 2) * 64
                pt = ps1.tile([128, 129], f32, tag="ps1",
                              name=f"tpk{s}_{c}_{h2}")
                nc.tensor.transpose(
                    pt[:, 0:64],
                    kst[pb:pb + 64, b // 2, :],
                    ident[pb:pb + 64, pb:pb + 64])
                nc.vector.tensor_copy(
                    kT[:, s, c, h2 * 64:(h2 + 1) * 64], pt[:, 0:64])

        def emit_vP_half(s, j):
            b = int(bt[s, j])
            c, h2 = j // 2, j % 2
            nc.vector.tensor_copy(
                vP[h2 * 64:(h2 + 1) * 64, s * NCH + c, 0:128],
                vst[(b % 2) * 64:(b % 2) * 64 + 64, b // 2, :])

        def emit_vP_ones(s, c):
            nc.vector.memset(vP[:, s * NCH + c, 128:129], 1.0)

        def emit_q_tr(p, qt):
            s, h = PAIRS[p]
            pt = ps1.tile([128, 129], f32, tag="ps1", name=f"tpq{p}_{qt}")
            nc.tensor.transpose(pt[:, 0:128], qstg[p][:, qt, :], ident[:, :])
            nc.vector.tensor_copy(
                qT[:, s, h, qt * 128:(qt + 1) * 128], pt[:, 0:128])

        def emit_fill(p, f):
            s, h = PAIRS[p]
            lo, hi, segs, diags = FILLS[f]
            sc = scp.tile([128, FILL], f32, tag="sc", name=f"sc{p}_{f}")
            for kt, a, b in segs:
                qcol = a - OFF[kt] + Q_LO[kt]
                nc.tensor.matmul(
                    sc[:, a - lo:b - lo],
                    kT[:, s, kt, :],
                    qT[:, s, h, qcol:qcol + (b - a)],
                    start=True, stop=True)
            nc.scalar.activation(
                es[p][:, lo:hi], sc[:, 0:hi - lo],
                mybir.ActivationFunctionType.Exp, scale=SCALE)
            for kt in diags:
                # zero the strictly-lower triangle of the diag panel
                dc = OFF[kt]
                nc.gpsimd.affine_select(
                    out=es[p][:, dc:dc + 128], in_=es[p][:, dc:dc + 128],
                    compare_op=mybir.AluOpType.is_ge, fill=0.0,
                    base=0, pattern=[[1, 128]], channel_multiplier=-1)

        def emit_pv(p, qt, pool=None):
            s, h = PAIRS[p]
            nch_q = NCH // 2 + 1 + qt   # kv chunks 0 .. 8+qt
            if pool is None:
                po = ps1.tile([128, 129], f32, tag="ps1", name=f"po{p}_{qt}")
            else:
                po = pool.tile([128, 129], f32, tag="sc", name=f"po{p}_{qt}")
            for c in range(nch_q):
                col = OFF[c] + qt * 128 - Q_LO[c]
                nc.tensor.matmul(
                    po[:, :],
                    es[p][:, col:col + 128],
                    vP[:, s * NCH + c, :],
                    start=(c == 0), stop=(c == nch_q - 1))
            rc = small.tile([128, 1], f32, tag="rc", name=f"rc{p}_{qt}")
            nc.vector.reciprocal(rc[:, :], po[:, 128:129])
            nc.vector.tensor_scalar_mul(
                ostg[s][:, qt, h * 128:(h + 1) * 128], po[:, 0:128], rc[:, :])

        def emit_out_dma(s, qt):
            nc.sync.dma_start(
                out=o_d[s * LQ + qt * 128:s * LQ + (qt + 1) * 128, :, :],
                in_=ostg[s][:, qt, :])

        # per-fill kT-chunk prerequisites (cumulative)
        chunks_needed = [max(kt for kt, _, _ in FILLS[f][2]) + 1
                         for f in range(NFILL)]

        # ---------------- load phase ----------------
        # DMA issue order gates the (shared, serial) DMA engines: K first
        # (kT feeds the first QK fills), then q0/q1, then V (vP is only
        # needed once PV of pair 0 starts, one pair-time later).
        emit_q_dma(0, split=True)
        # issue K quarters in order of earliest logical use by seq 0 so the
        # first fills' chunks land first (bt is baked into this program)
        def _qprio(qu):
            uses = [j // 2 for j in range(NBLK) if int(bt[0, j]) // 16 == qu]
            return min(uses) if uses else 99
        for quarter in sorted(range(4), key=_qprio):
            emit_kv_dma(k_d, kst, quarter * 1024, 1024)
        emit_q_dma(1)
        emit_kv_dma(v_d, vst, 0, LK)
        emit_kv_dma(v_d, vst, LK, LK)

        # ---------------- pair 0: fills + just-in-time transposes -------
        es[0] = espool.tile([128, TOTAL_COLS], f16, tag="es", name="es0")
        for qt in range(NQT):
            emit_q_tr(0, qt)
        done_chunks = 0
        for f in range(NFILL):
            while done_chunks < chunks_needed[f]:
                emit_kT_chunk(0, done_chunks)
                done_chunks += 1
            emit_fill(0, f)
            if 4 <= f <= 7:
                emit_q_tr(1, f - 4)
                emit_q_tr(1, f)
        while done_chunks < NCH:
            emit_kT_chunk(0, done_chunks)
            done_chunks += 1
        # vP for seq 0 (DVE-only block; V DMA has landed by now)
        for j in range(NBLK):
            emit_vP_half(0, j)
        for c in range(NCH):
            emit_vP_ones(0, c)

        # ---------------- steady pairs 1..7 ----------------
        LAST = len(PAIRS) - 1
        for p in range(1, len(PAIRS)):
            s, h = PAIRS[p]
            if p == 1:
                ostg[0] = opool.tile([128, NQT, GROUP * 128], f32,
                                     tag="ostg", name="ostg0")
            if p == 5:
                ostg[1] = opool.tile([128, NQT, GROUP * 128], f32,
                                     tag="ostg", name="ostg1")
            if 1 <= p <= 6:
                emit_q_dma(p + 1)
            es[p] = espool.tile([128, TOTAL_COLS], f16, tag="es",
                                name=f"es{p}")
            if p < LAST:
                # fills of pair p interleave PV batches of pair p-1
                for f in range(NFILL):
                    emit_fill(p, f)
                    if p in (1, 2) and f >= 3:
                        # seq-1 kT chunks: 6 per pair over pairs 1-2 (f3..f8)
                        emit_kT_chunk(1, (p - 1) * 6 + (f - 3))
                    if p == 3 and 5 <= f <= 8:
                        emit_kT_chunk(1, 12 + (f - 5))
                    if p in (4, 5) and f < 8:
                        # seq-1 vP copies, 2 per slot over pairs 4-5 (DVE)
                        base = (p - 4) * 16 + 2 * f
                        emit_vP_half(1, base)
                        emit_vP_half(1, base + 1)
                        emit_vP_ones(1, (p - 4) * 8 + f)
                    if 1 <= p <= 6 and f < NQT:
                        emit_q_tr(p + 1, f)
                    if f >= 1:
                        emit_pv(p - 1, f - 1)
                        if p == 5:
                            # seq-0 outputs: data has been ready since pair 4,
                            # so the DMA waits never block the SP sequencer
                            emit_out_dma(0, f - 1)
            else:
                # last pair: compress PV(p-1) into early slots and start
                # PV(p) as soon as its enabling fill (and diag mask) lands
                for f in range(NFILL):
                    emit_fill(p, f)
                    if f == 0:
                        emit_pv(p - 1, 0)
                    elif f <= 3:
                        emit_pv(p - 1, 2 * f - 1)
                        emit_pv(p - 1, 2 * f)
                    elif f == 4:
                        emit_pv(p - 1, 7)
                        # heads 0-2 of (seq1, qt7) are final after pair 6:
                        # write them now so the closing DMA only carries
                        # head 3's slice
                        nc.sync.dma_start(
                            out=o_d[LQ + 7 * 128:LQ + 8 * 128, 0:3, :],
                            in_=ostg[1][:, 7, 0:384])
                    # PV(p, qt) ready after fill 5 (qt0), 6 (qt1,2),
                    # 7 (qt3..5), 8 (qt6,7)
                    if f == 5:
                        emit_pv(p, 0)
                        emit_out_dma(1, 0)
                    elif f == 6:
                        emit_pv(p, 1)
                        emit_out_dma(1, 1)
                        emit_pv(p, 2)
                        emit_out_dma(1, 2)
                    elif f == 7:
                        emit_pv(p, 3)
                        emit_out_dma(1, 3)
                        emit_pv(p, 4)
                        emit_out_dma(1, 4)
                    elif f == 8:
                        emit_pv(p, 5)
                        emit_out_dma(1, 5)
                emit_pv(p, 6, pool=scp)
                emit_out_dma(1, 6)
                emit_pv(p, 7, pool=scp)
                nc.sync.dma_start(
                    out=o_d[LQ + 7 * 128:LQ + 8 * 128, 3, :],
                    in_=ostg[1][:, 7, 384:512])
            if p >= 2:
                del es[p - 2]

    nc.compile()
    return nc


def _get_program(bt: np.ndarray):
    key = bt.tobytes()
    if key not in _CACHE:
        _CACHE[key] = _build_program(bt)
    return _CACHE[key]


def kernel(q, k_cache, v_cache, cu_seqlens_q, cu_seqlens_k, block_tables,
           _want_trace=False):
    from concourse import bass_utils

    q = np.ascontiguousarray(np.asarray(q, dtype=np.float32))
    k_cache = np.ascontiguousarray(np.asarray(k_cache, dtype=np.float32))
    v_cache = np.ascontiguousarray(np.asarray(v_cache, dtype=np.float32))
    bt = np.asarray(block_tables, dtype=np.int32)

    assert q.shape == (NUM_SEQS * LQ, NUM_HEADS, HEAD_DIM)
    assert k_cache.shape == (NUM_SEQS * NBLK, BLOCK_SIZE, NUM_KV_HEADS,
                             HEAD_DIM)
    assert v_cache.shape == k_cache.shape
    assert bt.shape == (NUM_SEQS, NBLK)
    assert bt.min() >= 0

    nc = _get_program(bt)

    in_maps = []
    for core in range(NUM_KV_HEADS):
        in_maps.append({
            "q": np.ascontiguousarray(
                q[:, core * GROUP:(core + 1) * GROUP, :]),
            "k": np.ascontiguousarray(
                k_cache[:, :, core, :]).reshape(NUM_SEQS * LK, HEAD_DIM),
            "v": np.ascontiguousarray(
                v_cache[:, :, core, :]).reshape(NUM_SEQS * LK, HEAD_DIM),
        })

    res = bass_utils.run_bass_kernel_spmd(
        nc, in_maps, core_ids=list(range(NUM_KV_HEADS)),
        trace=_want_trace,
        **({"trace_cores": list(range(NUM_KV_HEADS)), "stitch_traces": True}
           if _want_trace else {}),
    )

    out = np.empty((NUM_SEQS * LQ, NUM_HEADS, HEAD_DIM), dtype=np.float32)
    for core in range(NUM_KV_HEADS):
        out[:, core * GROUP:(core + 1) * GROUP, :] = res.results[core]["out"]

    if _want_trace:
        return out, res
    return out
